# revision 10
# baseline (speedup 1.0000x reference)
"""GBST (segment_reduce) Trainium2 Bass kernel — nn_GBST_26061861552188.

kernel(**inputs) takes FULL unsharded inputs, returns FULL output
[4, 1024, 512] f32. 8 NeuronCores, data-parallel over (batch x seq-half).

Math (validated vs reference in numpy, rel err 2.8e-3 with bf16 tables):
  - emb gather + depthwise conv(K=4) + 1x1 proj fold into 4 per-shift
    lookup tables G_k = diag(conv_w[:,k]) emb @ proj_w.T (+C into G_0);
    gathers run as one-hot matmuls on PE.
  - per-position score z = h_proj . score_w folds the same way into 4
    256-entry tables g_k.
  - multi-scale block means of z via symmetric block-average matrices
    A_bs (PE matmuls); softmax over the 4 block scales.
  - the L x L consensus attention exp(q_i . q_j) factorizes exactly
    (scores live on the 3-simplex; exp approximated by a cubic) as
    psi(q_i)^T M psi(q_j) with 20 monomial features -> attention
    collapses to W = sum_j psi_j [q_j|1]^T, out = psi M W. No L x L.
  - fused blockrepr x score mixing + 4x mean pool become per-tile
    [120,30] matmuls with runtime-weighted pooling matrices.

SPMD: one program, 8 in_maps. Each core sees a "local" block of 18
tiles x 120 positions (its half, conv-extended ids) plus the "remote"
block (other half) for the global score/attention sums; duplicate and
pad positions are zeroed via an uploaded psi mask.
"""

import sys

for _p in ("/opt/trn_rl_repo", "/opt/trn_rl_repo/concourse"):
    if _p not in sys.path:
        sys.path.insert(0, _p)

import numpy as np
import ml_dtypes

K = 4
BLOCKS = (1, 2, 3, 4)
DS = 4
DIM = 512
NTOK = 256
N = 4096
L = 4104
B = 4
NC = 8

TS = 120            # positions per tile (divisible by lcm(1,2,3,4) and DS)
NT_LOC = 18         # local tiles per core (2160 positions)
NT = 36             # local + remote tiles in score pipeline
BLK = TS * NT_LOC   # 2160 positions per block
IDS_LEN = BLK + K - 1   # 2163 ids per block (conv lookahead)
IDSP = 2176         # padded ids row length
NF = 20             # simplex monomial features (degree <= 3)
DSR = TS // DS      # 30 ds rows per tile
OUTR = NT_LOC * DSR  # 540 output rows per core

BF16 = ml_dtypes.bfloat16

_CACHE = {}


def _bf(a):
    return np.asarray(a, np.float32).astype(BF16)


def _poly_M():
    """Bilinear matrix M with psi(q)^T M psi(k) ~= exp(q.k) on the simplex."""
    xs = np.linspace(0.0, 1.0, 2001)
    V = np.vander(xs, 4, increasing=True)
    coef, *_ = np.linalg.lstsq(V, np.exp(xs), rcond=None)
    rng = np.random.default_rng(0)

    def samp(n):
        e = rng.exponential(size=(n, 4))
        return (e / e.sum(1, keepdims=True)).astype(np.float64)

    Q = samp(4000)
    Kk = samp(4000)
    PQ = _monomials(Q)
    PK = _monomials(Kk)
    S = Q @ Kk.T
    E = sum(c * (S ** m) for m, c in enumerate(coef))
    M = np.linalg.pinv(PQ) @ E @ np.linalg.pinv(PK).T
    return M.astype(np.float32)


def _monomials(q):
    q1, q2, q3 = q[..., 0], q[..., 1], q[..., 2]
    one = np.ones_like(q1)
    return np.stack(
        [one, q1, q2, q3,
         q1 * q1, q1 * q2, q1 * q3, q2 * q2, q2 * q3, q3 * q3,
         q1 * q1 * q1, q1 * q1 * q2, q1 * q1 * q3, q1 * q2 * q2,
         q1 * q2 * q3, q1 * q3 * q3, q2 * q2 * q2, q2 * q2 * q3,
         q2 * q3 * q3, q3 * q3 * q3],
        axis=-1,
    )


def _pool_mats():
    """A_bs [120,120] block-average (for scores); Pt_bs [120,120]
    block-broadcast-sum / bs (for fused reprs); P4 [120,30] ds-pool*0.25."""
    at = np.zeros((TS, 4, TS), np.float32)
    pt = np.zeros((TS, 3, TS), np.float32)
    for ki, bs in enumerate(BLOCKS):
        m = np.zeros((TS, TS), np.float32)
        for j in range(TS // bs):
            m[j * bs:(j + 1) * bs, j * bs:(j + 1) * bs] = 1.0 / bs
        at[:, ki, :] = m
        if bs > 1:
            pt[:, ki - 1, :] = m  # block-sum / bs  (1/bs fused fold)
    p4 = np.zeros((TS, DSR), np.float32)
    for j in range(DSR):
        p4[j * DS:(j + 1) * DS, j] = 0.25
    return at, pt, p4


def _build_module():
    from concourse import bass, bacc, tile
    from concourse.bass import mybir

    f32 = mybir.dt.float32
    bf16 = mybir.dt.bfloat16
    nc = bacc.Bacc("TRN2", target_bir_lowering=False, debug=False,
                   num_devices=NC)

    def din(name, shape, dt=f32):
        return nc.dram_tensor(name, shape, dt, kind="ExternalInput")

    d_gt = din("gt", [128, 2, K, DIM], bf16)
    d_zt = din("ztab", [128, 2, K, 1], bf16)
    d_at = din("at", [TS, 4, TS], bf16)
    d_pt = din("pt", [TS, 3, TS], bf16)
    d_p4 = din("p4t", [TS, DSR], bf16)
    d_idt = din("idt", [TS, TS], f32)
    d_id1 = din("id1", [1, 1], f32)
    d_o128 = din("ones128", [1, 128], bf16)
    d_ocol = din("onescol", [TS, 1], f32)
    d_iota = din("iota", [128, 2], f32)
    d_mt = din("mt", [NF, NF], f32)
    d_ids = din("ids", [1, 2, IDSP], bf16)
    d_pm = din("pmask", [TS, NT], f32)
    d_out = nc.dram_tensor("out", [OUTR, DIM], f32, kind="ExternalOutput")

    EXP = mybir.ActivationFunctionType.Exp
    MUL = mybir.AluOpType.mult
    ADD = mybir.AluOpType.add
    ISEQ = mybir.AluOpType.is_equal
    X = mybir.AxisListType.X

    with tile.TileContext(nc) as tc:
        with tc.tile_pool(name="const", bufs=1) as cpool, \
             tc.tile_pool(name="sbuf", bufs=1) as spool, \
             tc.tile_pool(name="work", bufs=2) as wpool, \
             tc.tile_pool(name="pbig", bufs=2, space="PSUM") as pbig, \
             tc.tile_pool(name="pods", bufs=2, space="PSUM") as pods, \
             tc.tile_pool(name="poa", bufs=1, space="PSUM") as poa, \
             tc.tile_pool(name="psr", bufs=1, space="PSUM") as psr, \
             tc.tile_pool(name="pseq", bufs=2, space="PSUM") as pseq:

            def cload(dram, shape, dt, tag):
                t = cpool.tile(shape, dt, tag=tag)
                nc.sync.dma_start(t[:], dram.ap()[:])
                return t

            gt = cload(d_gt, [128, 2, K, DIM], bf16, "gt")
            ztab = cload(d_zt, [128, 2, K, 1], bf16, "ztab")
            at = cload(d_at, [TS, 4, TS], bf16, "at")
            pt = cload(d_pt, [TS, 3, TS], bf16, "pt")
            p4t = cload(d_p4, [TS, DSR], bf16, "p4t")
            idt = cload(d_idt, [TS, TS], f32, "idt")
            id1 = cload(d_id1, [1, 1], f32, "id1")
            o128 = cload(d_o128, [1, 128], bf16, "o128")
            ocol = cload(d_ocol, [TS, 1], f32, "ocol")
            iota = cload(d_iota, [128, 2], f32, "iota")
            mt = cload(d_mt, [NF, NF], f32, "mt")
            ids = cload(d_ids, [1, 2, IDSP], bf16, "ids")
            pmask = cload(d_pm, [TS, NT], f32, "pmask")

            # persistent SBUF buffers
            oh = spool.tile([128, 2, 2, IDSP], bf16)     # onehot [vc, blk, pos]
            h_sb = spool.tile([TS, NT_LOC, DIM], bf16)   # local h_proj
            ztl = spool.tile([TS, NT], bf16)             # z per tile col
            zrow = spool.tile([1, IDSP], f32)           # remote z row
            u_sb = spool.tile([TS, NT, 4], f32)
            den = spool.tile([TS, NT], f32)
            q_sb = spool.tile([TS, NT, 4], f32)
            psi = spool.tile([TS, NT, NF], f32)
            w_sb = spool.tile([NF, 4], f32)
            wq = spool.tile([NF, 4], f32)
            den2 = spool.tile([TS, NT_LOC], f32)
            fsum = spool.tile([TS, NT_LOC], f32)
            fp = spool.tile([TS, NT_LOC, 4], f32)
            F4 = spool.tile([TS, NT_LOC, 4, DSR], bf16)
            g2 = spool.tile([TS, 3, NT_LOC, DSR], bf16)

            # ---- 1. one-hot build: broadcast ids down 128 partitions, compare to iota
            SL = [(0, 512), (512, 512), (1024, 512), (1536, 512), (2048, 128)]
            for blk in range(2):
                for (s0, w) in SL:
                    idb = pbig.tile([128, 512], f32, tag="a")
                    nc.tensor.matmul(idb[:, :w], lhsT=o128[:, :],
                                     rhs=ids[:, blk, s0:s0 + w],
                                     start=True, stop=True)
                    for vc in range(2):
                        nc.vector.tensor_scalar(
                            out=oh[:, vc, blk, s0:s0 + w], in0=idb[:, :w],
                            scalar1=iota[:, vc:vc + 1], scalar2=None, op0=ISEQ)

            # ---- 2. gcp: local h tiles + local z (shared stationary one-hots)
            for t in range(NT_LOC):
                h_ps0 = pbig.tile([128, DIM], f32, tag="a")
                h_ps = h_ps0[:TS, :]
                z_ps = pseq.tile([TS, 1], f32, tag="s")
                first = True
                for k in range(K):
                    for vc in range(2):
                        lhs = oh[:, vc, 0, TS * t + k: TS * t + k + TS]
                        nc.tensor.matmul(h_ps[:], lhsT=lhs, rhs=gt[:, vc, k, :],
                                         start=first, stop=(k == K - 1 and vc == 1))
                        nc.tensor.matmul(z_ps[:], lhsT=lhs, rhs=ztab[:, vc, k, :],
                                         start=first, stop=(k == K - 1 and vc == 1))
                        first = False
                nc.scalar.copy(h_sb[:, t, :], h_ps[:])
                nc.scalar.copy(ztl[:, t:t + 1], z_ps[:])

            # ---- 3. remote z row (one-hot as moving operand), transpose to cols
            SLZ = [(0, 512), (512, 512), (1024, 512), (1536, 512), (2048, 115)]
            for (s0, w) in SLZ:
                zr_ps = pseq.tile([1, 512], f32, tag="s")
                first = True
                for k in range(K):
                    for vc in range(2):
                        nc.tensor.matmul(
                            zr_ps[:, :w], lhsT=ztab[:, vc, k, :],
                            rhs=oh[:, vc, 1, s0 + k: s0 + k + w],
                            start=first, stop=(k == K - 1 and vc == 1))
                        first = False
                nc.scalar.copy(zrow[:, s0:s0 + w], zr_ps[:, :w])
            zz = pseq.tile([TS, NT_LOC], f32, tag="s")
            for j in range(NT_LOC):
                nc.tensor.transpose(zz[:, j:j + 1],
                                    in_=zrow[:, TS * j: TS * j + TS],
                                    identity=id1[:, :])
            nc.scalar.copy(ztl[:, NT_LOC:NT], zz[:])

            # ---- 4. block-scale scores: A_bs pooling matmuls, exp, softmax, psi
            sraw = psr.tile([TS, NT * 4], f32)
            for k in range(4):
                for t in range(NT):
                    nc.tensor.matmul(sraw[:, 4 * t + k: 4 * t + k + 1],
                                     lhsT=at[:, k, :], rhs=ztl[:, t:t + 1],
                                     start=True, stop=True)
            nc.scalar.activation(u_sb[:].rearrange("p t k -> p (t k)"), sraw[:],
                                 EXP, bias=0.0, scale=1.0)
            nc.vector.tensor_reduce(den[:], u_sb[:], axis=X, op=ADD)
            nc.vector.reciprocal(den[:], den[:])
            nc.vector.tensor_tensor(
                q_sb[:], u_sb[:],
                den[:].unsqueeze(2).broadcast_to([TS, NT, 4]), op=MUL)

            nc.vector.memset(psi[:, :, 0:1], 1.0)
            nc.vector.tensor_copy(psi[:, :, 1:4], q_sb[:, :, 0:3])
            deg2 = [(4, 1, 1), (5, 1, 2), (6, 1, 3), (7, 2, 2), (8, 2, 3),
                    (9, 3, 3)]
            deg3 = [(10, 1, 4), (11, 1, 5), (12, 1, 6), (13, 1, 7), (14, 1, 8),
                    (15, 1, 9), (16, 2, 7), (17, 2, 8), (18, 2, 9), (19, 3, 9)]
            for (o, a, bb) in deg2 + deg3:
                nc.vector.tensor_tensor(psi[:, :, o:o + 1], psi[:, :, a:a + 1],
                                        psi[:, :, bb:bb + 1], op=MUL)
            nc.vector.tensor_tensor(
                psi[:], psi[:],
                pmask[:].unsqueeze(2).broadcast_to([TS, NT, NF]), op=MUL)

            # ---- 5. W = sum_j psi_j [q_j(1:3) | 1]   (both blocks)
            wps = pseq.tile([NF, 4], f32, tag="s")
            for t in range(NT):
                nc.tensor.matmul(wps[:, 0:3], lhsT=psi[:, t, :],
                                 rhs=q_sb[:, t, 0:3],
                                 start=(t == 0), stop=(t == NT - 1))
                nc.tensor.matmul(wps[:, 3:4], lhsT=psi[:, t, :],
                                 rhs=ocol[:, :],
                                 start=(t == 0), stop=(t == NT - 1))
            nc.vector.tensor_copy(w_sb[:], wps[:])
            wqp = pseq.tile([NF, 4], f32, tag="s")
            nc.tensor.matmul(wqp[:], lhsT=mt[:, :], rhs=w_sb[:],
                             start=True, stop=True)
            nc.vector.tensor_copy(wq[:], wqp[:])

            # ---- 6. out_aug = psi @ (M W) for local tiles (via PE transpose)
            oa = poa.tile([TS, NT_LOC * 4], f32)
            for t in range(NT_LOC):
                pT = pseq.tile([NF, TS], f32, tag="s")
                nc.tensor.transpose(pT[:], in_=psi[:, t, :], identity=idt[:, :])
                pTs = wpool.tile([NF, TS], f32, tag="pTs")
                nc.scalar.copy(pTs[:], pT[:])
                nc.tensor.matmul(oa[:, 4 * t: 4 * t + 4], lhsT=pTs[:],
                                 rhs=wq[:], start=True, stop=True)

            # ---- 7. consensus scores f (f4 = 1 - f1 - f2 - f3)
            oa3 = oa[:].rearrange("p (t k) -> p t k", k=4)
            nc.vector.tensor_scalar_max(den2[:], oa3[:, :, 3], 1e-20)
            nc.vector.reciprocal(den2[:], den2[:])
            nc.vector.tensor_tensor(
                fp[:, :, 0:3], oa3[:, :, 0:3],
                den2[:].unsqueeze(2).broadcast_to([TS, NT_LOC, 3]), op=MUL)
            nc.vector.tensor_reduce(fsum[:], fp[:, :, 0:3], axis=X, op=ADD)
            nc.vector.tensor_scalar(out=fp[:, :, 3:4].rearrange("p t k -> p (t k)"),
                                    in0=fsum[:], scalar1=-1.0, scalar2=1.0,
                                    op0=MUL, op1=ADD)

            # ---- 8. runtime pooling matrices F_k = p4 * f'_k; Gk = Pt_bs @ F_k
            nc.vector.tensor_tensor(
                F4[:],
                fp[:].unsqueeze(3).broadcast_to([TS, NT_LOC, 4, DSR]),
                p4t[:].unsqueeze(1).unsqueeze(1).broadcast_to(
                    [TS, NT_LOC, 4, DSR]),
                op=MUL)
            for kk in range(3):
                ga0 = pbig.tile([128, 510], f32, tag="a")
                ga = ga0[:TS, :]
                gb = pseq.tile([TS, DSR], f32, tag="s")
                for t in range(NT_LOC):
                    out_ap = ga[:, DSR * t: DSR * t + DSR] if t < 17 else gb[:]
                    nc.tensor.matmul(out_ap, lhsT=pt[:, kk, :],
                                     rhs=F4[:, t, kk + 1, :],
                                     start=True, stop=True)
                nc.scalar.copy(
                    g2[:, kk, 0:17, :],
                    ga[:].rearrange("p (t c) -> p t c", c=DSR))
                nc.scalar.copy(g2[:, kk, 17, :], gb[:])

            # ---- 9. fused mixing + 4x downsample pool, DMA out
            for t in range(NT_LOC):
                ods = pods.tile([DSR, DIM], f32, tag="ods")
                nc.tensor.matmul(ods[:], lhsT=F4[:, t, 0, :], rhs=h_sb[:, t, :],
                                 start=True, stop=False)
                for kk in range(3):
                    nc.tensor.matmul(ods[:], lhsT=g2[:, kk, t, :],
                                     rhs=h_sb[:, t, :],
                                     start=False, stop=(kk == 2))
                osb = wpool.tile([DSR, DIM], f32, tag="osb")
                nc.scalar.copy(osb[:], ods[:])
                nc.sync.dma_start(d_out.ap()[DSR * t: DSR * t + DSR, :], osb[:])

    nc.compile()
    return nc


def _host_prep(emb, conv_w, conv_b, proj_w, proj_b, score_w, score_b):
    G = np.stack([(emb * conv_w[:, k][None, :]) @ proj_w.T for k in range(K)])
    C = conv_b @ proj_w.T + proj_b
    G[0] += C
    g = G @ score_w                      # [4, 256]
    gt = np.zeros((128, 2, K, DIM), np.float32)
    zt = np.zeros((128, 2, K, 1), np.float32)
    for vc in range(2):
        for k in range(K):
            gt[:, vc, k, :] = G[k][128 * vc:128 * vc + 128]
            zt[:, vc, k, 0] = g[k][128 * vc:128 * vc + 128]
    at, pt, p4 = _pool_mats()
    M = _poly_M()
    iota = np.stack([np.arange(128, dtype=np.float32),
                     np.arange(128, 256, dtype=np.float32)], axis=1)
    consts = {
        "gt": _bf(gt), "ztab": _bf(zt), "at": _bf(at), "pt": _bf(pt),
        "p4t": _bf(p4), "idt": np.eye(TS, dtype=np.float32),
        "id1": np.ones((1, 1), np.float32), "ones128": _bf(np.ones((1, 128))),
        "onescol": np.ones((TS, 1), np.float32), "iota": iota,
        "mt": np.ascontiguousarray(M.T),
    }
    return consts, np.float32(score_b)


def _core_inputs(x_row, hi):
    """ids [1,2,IDSP] bf16 and pmask [TS,NT] f32 for core half hi."""
    idsp = np.full(L + K - 1, -1.0, np.float32)
    idsp[:N] = x_row.astype(np.float32)

    def block(start):
        out = np.full(IDSP, -1.0, np.float32)
        lo = start
        hhi = min(start + IDS_LEN, L + K - 1)
        out[:hhi - lo] = idsp[lo:hhi]
        return out

    o_loc = 0 if hi == 0 else L - BLK          # 0 or 1944? no: 2040
    o_loc = 0 if hi == 0 else 2040
    o_rem = 2040 if hi == 0 else 0
    ids = np.stack([block(o_loc), block(o_rem)])[None]  # [1,2,IDSP]

    pm = np.zeros((TS, NT), np.float32)
    for j in range(NT):
        base = (o_loc if j < NT_LOC else o_rem) + TS * (j % NT_LOC)
        gpos = base + np.arange(TS)
        valid = gpos < L
        if hi == 0 and j == NT_LOC:          # remote tile at 2040 duplicates local t=17
            valid &= False
        if hi == 1 and j == NT - 1:          # remote tile at 2040 duplicates local t=0
            valid &= False
        pm[:, j] = valid.astype(np.float32)
    return _bf(ids), pm


def kernel(x, emb, conv_w, conv_b, proj_w, proj_b, score_w, score_b):
    from concourse import bass_utils

    x = np.asarray(x)
    emb = np.asarray(emb, np.float32)
    conv_w = np.asarray(conv_w, np.float32)
    conv_b = np.asarray(conv_b, np.float32)
    proj_w = np.asarray(proj_w, np.float32)
    proj_b = np.asarray(proj_b, np.float32)
    score_w = np.asarray(score_w, np.float32)
    score_b = np.float32(np.asarray(score_b))

    if "nc" not in _CACHE:
        _CACHE["nc"] = _build_module()
    nc = _CACHE["nc"]

    consts, sb = _host_prep(emb, conv_w, conv_b, proj_w, proj_b,
                            score_w, score_b)
    # score_b is folded as exp bias -> bake into ztab? No: it is a bias on
    # scores_raw. We add it on host into the A_bs result via ztab offset:
    # scores_raw = A_bs z + score_b. Instead fold into z: z' = z + score_b
    # would shift all scales equally -> softmax invariant? No: A_bs averages
    # z, so adding score_b to every z entry adds score_b to every score --
    # softmax over k is invariant to a common shift. pad positions get
    # masked anyway. So score_b can be DROPPED entirely.
    del sb

    in_maps = []
    for c in range(NC):
        bi, hi = divmod(c, 2)
        ids, pm = _core_inputs(x[bi], hi)
        m = dict(consts)
        m["ids"] = ids
        m["pmask"] = pm
        in_maps.append(m)

    res = bass_utils.run_bass_kernel_spmd(
        nc, in_maps, core_ids=list(range(NC)), trace=_CACHE.get("trace", False))
    _CACHE["last_exec_ns"] = res.exec_time_ns

    out = np.empty((B, N // DS, DIM), np.float32)
    for c in range(NC):
        bi, hi = divmod(c, 2)
        r = res.results[c]["out"]
        if hi == 0:
            out[bi, 0:540] = r[0:540]
        else:
            out[bi, 540:1024] = r[30:514]
    return out


# revision 11
# speedup vs baseline: 1.1889x; 1.1889x over previous
"""GBST (segment_reduce) Trainium2 Bass kernel — nn_GBST_26061861552188.

kernel(**inputs) takes FULL unsharded inputs, returns FULL output
[4, 1024, 512] f32. 8 NeuronCores, data-parallel over (batch x seq-half).

Math (validated vs reference in numpy, rel err 2.8e-3 with bf16 tables):
  - emb gather + depthwise conv(K=4) + 1x1 proj fold into 4 per-shift
    lookup tables G_k = diag(conv_w[:,k]) emb @ proj_w.T (+C into G_0);
    gathers run as one-hot matmuls on PE.
  - per-position score z = h_proj . score_w folds the same way into 4
    256-entry tables g_k.
  - multi-scale block means of z via symmetric block-average matrices
    A_bs (PE matmuls); softmax over the 4 block scales.
  - the L x L consensus attention exp(q_i . q_j) factorizes exactly
    (scores live on the 3-simplex; exp approximated by a cubic) as
    psi(q_i)^T M psi(q_j) with 20 monomial features -> attention
    collapses to W = sum_j psi_j [q_j|1]^T, out = psi M W. No L x L.
  - fused blockrepr x score mixing + 4x mean pool become per-tile
    [120,30] matmuls with runtime-weighted pooling matrices.

SPMD: one program, 8 in_maps. Each core sees a "local" block of 18
tiles x 120 positions (its half, conv-extended ids) plus the "remote"
block (other half) for the global score/attention sums; duplicate and
pad positions are zeroed via an uploaded psi mask.
"""

import sys

for _p in ("/opt/trn_rl_repo", "/opt/trn_rl_repo/concourse"):
    if _p not in sys.path:
        sys.path.insert(0, _p)

import numpy as np
import ml_dtypes

K = 4
BLOCKS = (1, 2, 3, 4)
DS = 4
DIM = 512
NTOK = 256
N = 4096
L = 4104
B = 4
NC = 8

TS = 120            # positions per tile (divisible by lcm(1,2,3,4) and DS)
NT_LOC = 18         # local tiles per core (2160 positions)
NT = 36             # local + remote tiles in score pipeline
BLK = TS * NT_LOC   # 2160 positions per block
IDS_LEN = BLK + K - 1   # 2163 ids per block (conv lookahead)
IDSP = 2176         # padded ids row length
NF = 20             # simplex monomial features (degree <= 3)
DSR = TS // DS      # 30 ds rows per tile
OUTR = NT_LOC * DSR  # 540 output rows per core

BF16 = ml_dtypes.bfloat16

_CACHE = {}


def _bf(a):
    return np.asarray(a, np.float32).astype(BF16)


def _poly_M():
    """Bilinear matrix M with psi(q)^T M psi(k) ~= exp(q.k) on the simplex."""
    xs = np.linspace(0.0, 1.0, 2001)
    V = np.vander(xs, 4, increasing=True)
    coef, *_ = np.linalg.lstsq(V, np.exp(xs), rcond=None)
    rng = np.random.default_rng(0)

    def samp(n):
        e = rng.exponential(size=(n, 4))
        return (e / e.sum(1, keepdims=True)).astype(np.float64)

    Q = samp(4000)
    Kk = samp(4000)
    PQ = _monomials(Q)
    PK = _monomials(Kk)
    S = Q @ Kk.T
    E = sum(c * (S ** m) for m, c in enumerate(coef))
    M = np.linalg.pinv(PQ) @ E @ np.linalg.pinv(PK).T
    return M.astype(np.float32)


def _monomials(q):
    q1, q2, q3 = q[..., 0], q[..., 1], q[..., 2]
    one = np.ones_like(q1)
    return np.stack(
        [one, q1, q2, q3,
         q1 * q1, q1 * q2, q1 * q3, q2 * q2, q2 * q3, q3 * q3,
         q1 * q1 * q1, q1 * q1 * q2, q1 * q1 * q3, q1 * q2 * q2,
         q1 * q2 * q3, q1 * q3 * q3, q2 * q2 * q2, q2 * q2 * q3,
         q2 * q3 * q3, q3 * q3 * q3],
        axis=-1,
    )


def _pool_mats():
    """A_bs [120,120] block-average (for scores); Pt_bs [120,120]
    block-broadcast-sum / bs (for fused reprs); P4 [120,30] ds-pool*0.25."""
    at = np.zeros((TS, 4, TS), np.float32)
    pt = np.zeros((TS, 3, TS), np.float32)
    for ki, bs in enumerate(BLOCKS):
        m = np.zeros((TS, TS), np.float32)
        for j in range(TS // bs):
            m[j * bs:(j + 1) * bs, j * bs:(j + 1) * bs] = 1.0 / bs
        at[:, ki, :] = m
        if bs > 1:
            pt[:, ki - 1, :] = m  # block-sum / bs  (1/bs fused fold)
    p4 = np.zeros((TS, DSR), np.float32)
    for j in range(DSR):
        p4[j * DS:(j + 1) * DS, j] = 0.25
    return at, pt, p4


def _build_module():
    from concourse import bass, bacc, tile
    from concourse.bass import mybir

    f32 = mybir.dt.float32
    bf16 = mybir.dt.bfloat16
    nc = bacc.Bacc("TRN2", target_bir_lowering=False, debug=False,
                   num_devices=NC)

    def din(name, shape, dt=f32):
        return nc.dram_tensor(name, shape, dt, kind="ExternalInput")

    d_gt = din("gt", [128, 2, K, DIM], bf16)
    d_zt = din("ztab", [128, 2, K, 1], bf16)
    d_at = din("at", [TS, 4, TS], bf16)
    d_pt = din("pt", [TS, 3, TS], bf16)
    d_p4 = din("p4t", [TS, DSR], bf16)
    d_idt = din("idt", [TS, TS], f32)
    d_id1 = din("id1", [1, 1], f32)
    d_o128 = din("ones128", [1, 128], bf16)
    d_ocol = din("onescol", [TS, 1], f32)
    d_iota = din("iota", [128, 2], f32)
    d_mt = din("mt", [NF, NF], f32)
    d_ids = din("ids", [1, 2, IDSP], bf16)
    d_pm = din("pmask", [TS, NT], f32)
    d_out = nc.dram_tensor("out", [OUTR, DIM], f32, kind="ExternalOutput")

    EXP = mybir.ActivationFunctionType.Exp
    MUL = mybir.AluOpType.mult
    ADD = mybir.AluOpType.add
    ISEQ = mybir.AluOpType.is_equal
    X = mybir.AxisListType.X

    with tile.TileContext(nc) as tc:
        with tc.tile_pool(name="const", bufs=1) as cpool, \
             tc.tile_pool(name="sbuf", bufs=1) as spool, \
             tc.tile_pool(name="work", bufs=2) as wpool, \
             tc.tile_pool(name="pbig", bufs=2, space="PSUM") as pbig, \
             tc.tile_pool(name="pods", bufs=2, space="PSUM") as pods, \
             tc.tile_pool(name="poa", bufs=1, space="PSUM") as poa, \
             tc.tile_pool(name="psr", bufs=1, space="PSUM") as psr, \
             tc.tile_pool(name="pseq", bufs=2, space="PSUM") as pseq:

            def cload(dram, shape, dt, tag, eng=None):
                t = cpool.tile(shape, dt, tag=tag)
                (eng or nc.sync).dma_start(t[:], dram.ap()[:])
                return t

            ids = cload(d_ids, [1, 2, IDSP], bf16, "ids")
            iota = cload(d_iota, [128, 2], f32, "iota")
            o128 = cload(d_o128, [1, 128], bf16, "o128")
            ztab = cload(d_zt, [128, 2, K, 1], bf16, "ztab")
            id1 = cload(d_id1, [1, 1], f32, "id1")
            gt = cload(d_gt, [128, 2, K, DIM], bf16, "gt", nc.gpsimd)
            at = cload(d_at, [TS, 4, TS], bf16, "at", nc.gpsimd)
            pt = cload(d_pt, [TS, 3, TS], bf16, "pt", nc.gpsimd)
            p4t = cload(d_p4, [TS, DSR], bf16, "p4t")
            idt = cload(d_idt, [TS, TS], f32, "idt", nc.gpsimd)
            ocol = cload(d_ocol, [TS, 1], f32, "ocol")
            mt = cload(d_mt, [NF, NF], f32, "mt")
            pmask = cload(d_pm, [TS, NT], f32, "pmask")

            # persistent SBUF buffers
            oh = spool.tile([128, 2, 2, IDSP], bf16)     # onehot [vc, blk, pos]
            h_sb = spool.tile([TS, NT_LOC, DIM], bf16)   # local h_proj
            ztl = spool.tile([TS, NT], bf16)             # z per tile col
            zrow = spool.tile([1, 2, IDSP], f32)           # remote z row
            u_sb = spool.tile([TS, NT, 4], f32)
            den = spool.tile([TS, NT], f32)
            q_sb = spool.tile([TS, NT, 4], f32)
            psi = spool.tile([TS, NT, NF], f32)
            w_sb = spool.tile([NF, 4], f32)
            wq = spool.tile([NF, 4], f32)
            den2 = spool.tile([TS, NT_LOC], f32)
            fsum = spool.tile([TS, NT_LOC], f32)
            fp = spool.tile([TS, NT_LOC, 4], f32)
            F4 = spool.tile([TS, NT_LOC, 4, DSR], bf16)
            g2 = spool.tile([TS, 3, NT_LOC, DSR], bf16)

            # ---- 1. one-hot build: broadcast ids down 128 partitions, compare to iota
            SL = [(0, 512), (512, 512), (1024, 512), (1536, 512), (2048, 128)]
            for blk in range(2):
                for (s0, w) in SL:
                    idb = pbig.tile([128, 512], f32, tag="a")
                    nc.tensor.matmul(idb[:, :w], lhsT=o128[:, :],
                                     rhs=ids[:, blk, s0:s0 + w],
                                     start=True, stop=True)
                    for vc in range(2):
                        nc.vector.tensor_scalar(
                            out=oh[:, vc, blk, s0:s0 + w], in0=idb[:, :w],
                            scalar1=iota[:, vc:vc + 1], scalar2=None, op0=ISEQ)

            # ---- 2. gcp: local h tiles + local z (shared stationary one-hots)
            for t in range(NT_LOC):
                h_ps0 = pbig.tile([128, DIM], f32, tag="a")
                h_ps = h_ps0[:TS, :]
                first = True
                for k in range(K):
                    for vc in range(2):
                        lhs = oh[:, vc, 0, TS * t + k: TS * t + k + TS]
                        nc.tensor.matmul(h_ps[:], lhsT=lhs, rhs=gt[:, vc, k, :],
                                         start=first, stop=(k == K - 1 and vc == 1))
                        first = False
                nc.scalar.copy(h_sb[:, t, :], h_ps[:])

            # ---- 3. z rows (one-hot as moving operand), transpose to cols
            SLZ = [(0, 512), (512, 512), (1024, 512), (1536, 512), (2048, 115)]
            for blk in range(2):
                for (s0, w) in SLZ:
                    zr_ps = pseq.tile([1, 512], f32, tag="s")
                    first = True
                    for k in range(K):
                        for vc in range(2):
                            nc.tensor.matmul(
                                zr_ps[:, :w], lhsT=ztab[:, vc, k, :],
                                rhs=oh[:, vc, blk, s0 + k: s0 + k + w],
                                start=first, stop=(k == K - 1 and vc == 1))
                            first = False
                    nc.scalar.copy(zrow[:, blk, s0:s0 + w], zr_ps[:, :w])
            zz = pseq.tile([TS, NT], f32, tag="s")
            for blk in range(2):
                for j in range(NT_LOC):
                    nc.tensor.transpose(
                        zz[:, NT_LOC * blk + j: NT_LOC * blk + j + 1],
                        in_=zrow[:, blk, TS * j: TS * j + TS],
                        identity=id1[:, :])
            nc.scalar.copy(ztl[:], zz[:])

            # ---- 4. block-scale scores: A_bs pooling matmuls, exp, softmax, psi
            sraw = psr.tile([TS, 4, NT], f32)
            for k in range(4):
                nc.tensor.matmul(sraw[:, k, :], lhsT=at[:, k, :], rhs=ztl[:],
                                 start=True, stop=True)
            nc.scalar.activation(u_sb[:].rearrange("p t k -> p k t"), sraw[:],
                                 EXP, bias=0.0, scale=1.0)
            nc.vector.tensor_reduce(den[:], u_sb[:], axis=X, op=ADD)
            nc.vector.reciprocal(den[:], den[:])
            nc.vector.tensor_tensor(
                q_sb[:], u_sb[:],
                den[:].unsqueeze(2).broadcast_to([TS, NT, 4]), op=MUL)

            nc.vector.memset(psi[:, :, 0:1], 1.0)
            nc.vector.tensor_copy(psi[:, :, 1:4], q_sb[:, :, 0:3])
            deg2 = [(4, 1, 1), (5, 1, 2), (6, 1, 3), (7, 2, 2), (8, 2, 3),
                    (9, 3, 3)]
            deg3 = [(10, 1, 4), (11, 1, 5), (12, 1, 6), (13, 1, 7), (14, 1, 8),
                    (15, 1, 9), (16, 2, 7), (17, 2, 8), (18, 2, 9), (19, 3, 9)]
            for (o, a, bb) in deg2 + deg3:
                nc.vector.tensor_tensor(psi[:, :, o:o + 1], psi[:, :, a:a + 1],
                                        psi[:, :, bb:bb + 1], op=MUL)
            nc.vector.tensor_tensor(
                psi[:], psi[:],
                pmask[:].unsqueeze(2).broadcast_to([TS, NT, NF]), op=MUL)

            # ---- 5. W = sum_j psi_j [q_j(1:3) | 1]   (both blocks)
            vaug = spool.tile([TS, NT, 4], f32)
            nc.vector.tensor_copy(vaug[:, :, 0:3], q_sb[:, :, 0:3])
            nc.vector.memset(vaug[:, :, 3:4], 1.0)
            wps = pseq.tile([NF, 4], f32, tag="s")
            for t in range(NT):
                nc.tensor.matmul(wps[:], lhsT=psi[:, t, :],
                                 rhs=vaug[:, t, :],
                                 start=(t == 0), stop=(t == NT - 1))
            nc.vector.tensor_copy(w_sb[:], wps[:])
            wqp = pseq.tile([NF, 4], f32, tag="s")
            nc.tensor.matmul(wqp[:], lhsT=mt[:, :], rhs=w_sb[:],
                             start=True, stop=True)
            nc.vector.tensor_copy(wq[:], wqp[:])

            # ---- 6. out_aug = psi @ (M W) for local tiles (via PE transpose)
            oa = poa.tile([TS, NT_LOC * 4], f32)
            for t in range(NT_LOC):
                pT = pseq.tile([NF, TS], f32, tag="s")
                nc.tensor.transpose(pT[:], in_=psi[:, t, :], identity=idt[:, :])
                pTs = wpool.tile([NF, TS], f32, tag="pTs")
                nc.scalar.copy(pTs[:], pT[:])
                nc.tensor.matmul(oa[:, 4 * t: 4 * t + 4], lhsT=pTs[:],
                                 rhs=wq[:], start=True, stop=True)

            # ---- 7. consensus scores f (f4 = 1 - f1 - f2 - f3)
            oa3 = oa[:].rearrange("p (t k) -> p t k", k=4)
            nc.vector.tensor_scalar_max(den2[:], oa3[:, :, 3], 1e-20)
            nc.vector.reciprocal(den2[:], den2[:])
            nc.vector.tensor_tensor(
                fp[:, :, 0:3], oa3[:, :, 0:3],
                den2[:].unsqueeze(2).broadcast_to([TS, NT_LOC, 3]), op=MUL)
            nc.vector.tensor_reduce(fsum[:], fp[:, :, 0:3], axis=X, op=ADD)
            nc.vector.tensor_scalar(out=fp[:, :, 3:4].rearrange("p t k -> p (t k)"),
                                    in0=fsum[:], scalar1=-1.0, scalar2=1.0,
                                    op0=MUL, op1=ADD)

            # ---- 8. runtime pooling matrices F_k = p4 * f'_k; Gk = Pt_bs @ F_k
            nc.vector.tensor_tensor(
                F4[:],
                fp[:].unsqueeze(3).broadcast_to([TS, NT_LOC, 4, DSR]),
                p4t[:].unsqueeze(1).unsqueeze(1).broadcast_to(
                    [TS, NT_LOC, 4, DSR]),
                op=MUL)
            for kk in range(3):
                ga0 = pbig.tile([128, 510], f32, tag="a")
                ga = ga0[:TS, :]
                gb = pseq.tile([TS, DSR], f32, tag="s")
                for t in range(NT_LOC):
                    out_ap = ga[:, DSR * t: DSR * t + DSR] if t < 17 else gb[:]
                    nc.tensor.matmul(out_ap, lhsT=pt[:, kk, :],
                                     rhs=F4[:, t, kk + 1, :],
                                     start=True, stop=True)
                nc.scalar.copy(
                    g2[:, kk, 0:17, :],
                    ga[:].rearrange("p (t c) -> p t c", c=DSR))
                nc.scalar.copy(g2[:, kk, 17, :], gb[:])

            # ---- 9. fused mixing + 4x downsample pool, DMA out
            for t in range(NT_LOC):
                ods = pods.tile([DSR, DIM], f32, tag="ods")
                nc.tensor.matmul(ods[:], lhsT=F4[:, t, 0, :], rhs=h_sb[:, t, :],
                                 start=True, stop=False)
                for kk in range(3):
                    nc.tensor.matmul(ods[:], lhsT=g2[:, kk, t, :],
                                     rhs=h_sb[:, t, :],
                                     start=False, stop=(kk == 2))
                osb = wpool.tile([DSR, DIM], f32, tag="osb")
                nc.scalar.copy(osb[:], ods[:])
                nc.sync.dma_start(d_out.ap()[DSR * t: DSR * t + DSR, :], osb[:])

    nc.compile()
    return nc


def _host_prep(emb, conv_w, conv_b, proj_w, proj_b, score_w, score_b):
    G = np.stack([(emb * conv_w[:, k][None, :]) @ proj_w.T for k in range(K)])
    C = conv_b @ proj_w.T + proj_b
    G[0] += C
    g = G @ score_w                      # [4, 256]
    gt = np.zeros((128, 2, K, DIM), np.float32)
    zt = np.zeros((128, 2, K, 1), np.float32)
    for vc in range(2):
        for k in range(K):
            gt[:, vc, k, :] = G[k][128 * vc:128 * vc + 128]
            zt[:, vc, k, 0] = g[k][128 * vc:128 * vc + 128]
    at, pt, p4 = _pool_mats()
    M = _poly_M()
    iota = np.stack([np.arange(128, dtype=np.float32),
                     np.arange(128, 256, dtype=np.float32)], axis=1)
    consts = {
        "gt": _bf(gt), "ztab": _bf(zt), "at": _bf(at), "pt": _bf(pt),
        "p4t": _bf(p4), "idt": np.eye(TS, dtype=np.float32),
        "id1": np.ones((1, 1), np.float32), "ones128": _bf(np.ones((1, 128))),
        "onescol": np.ones((TS, 1), np.float32), "iota": iota,
        "mt": np.ascontiguousarray(M.T),
    }
    return consts, np.float32(score_b)


def _core_inputs(x_row, hi):
    """ids [1,2,IDSP] bf16 and pmask [TS,NT] f32 for core half hi."""
    idsp = np.full(L + K - 1, -1.0, np.float32)
    idsp[:N] = x_row.astype(np.float32)

    def block(start):
        out = np.full(IDSP, -1.0, np.float32)
        lo = start
        hhi = min(start + IDS_LEN, L + K - 1)
        out[:hhi - lo] = idsp[lo:hhi]
        return out

    o_loc = 0 if hi == 0 else L - BLK          # 0 or 1944? no: 2040
    o_loc = 0 if hi == 0 else 2040
    o_rem = 2040 if hi == 0 else 0
    ids = np.stack([block(o_loc), block(o_rem)])[None]  # [1,2,IDSP]

    pm = np.zeros((TS, NT), np.float32)
    for j in range(NT):
        base = (o_loc if j < NT_LOC else o_rem) + TS * (j % NT_LOC)
        gpos = base + np.arange(TS)
        valid = gpos < L
        if hi == 0 and j == NT_LOC:          # remote tile at 2040 duplicates local t=17
            valid &= False
        if hi == 1 and j == NT - 1:          # remote tile at 2040 duplicates local t=0
            valid &= False
        pm[:, j] = valid.astype(np.float32)
    return _bf(ids), pm


def kernel(x, emb, conv_w, conv_b, proj_w, proj_b, score_w, score_b):
    from concourse import bass_utils

    x = np.asarray(x)
    emb = np.asarray(emb, np.float32)
    conv_w = np.asarray(conv_w, np.float32)
    conv_b = np.asarray(conv_b, np.float32)
    proj_w = np.asarray(proj_w, np.float32)
    proj_b = np.asarray(proj_b, np.float32)
    score_w = np.asarray(score_w, np.float32)
    score_b = np.float32(np.asarray(score_b))

    if "nc" not in _CACHE:
        _CACHE["nc"] = _build_module()
    nc = _CACHE["nc"]

    consts, sb = _host_prep(emb, conv_w, conv_b, proj_w, proj_b,
                            score_w, score_b)
    # score_b is folded as exp bias -> bake into ztab? No: it is a bias on
    # scores_raw. We add it on host into the A_bs result via ztab offset:
    # scores_raw = A_bs z + score_b. Instead fold into z: z' = z + score_b
    # would shift all scales equally -> softmax invariant? No: A_bs averages
    # z, so adding score_b to every z entry adds score_b to every score --
    # softmax over k is invariant to a common shift. pad positions get
    # masked anyway. So score_b can be DROPPED entirely.
    del sb

    in_maps = []
    for c in range(NC):
        bi, hi = divmod(c, 2)
        ids, pm = _core_inputs(x[bi], hi)
        m = dict(consts)
        m["ids"] = ids
        m["pmask"] = pm
        in_maps.append(m)

    res = bass_utils.run_bass_kernel_spmd(
        nc, in_maps, core_ids=list(range(NC)), trace=_CACHE.get("trace", False))
    _CACHE["last_exec_ns"] = res.exec_time_ns

    out = np.empty((B, N // DS, DIM), np.float32)
    for c in range(NC):
        bi, hi = divmod(c, 2)
        r = res.results[c]["out"]
        if hi == 0:
            out[bi, 0:540] = r[0:540]
        else:
            out[bi, 540:1024] = r[30:514]
    return out


# revision 13
# speedup vs baseline: 1.5613x; 1.3132x over previous
"""GBST (segment_reduce) Trainium2 Bass kernel — nn_GBST_26061861552188.

kernel(**inputs) takes FULL unsharded inputs, returns FULL output
[4, 1024, 512] f32. 8 NeuronCores, data-parallel over (batch x seq-half).

Math (validated vs reference in numpy, rel err 2.8e-3 with bf16 tables):
  - emb gather + depthwise conv(K=4) + 1x1 proj fold into 4 per-shift
    lookup tables G_k = diag(conv_w[:,k]) emb @ proj_w.T (+C into G_0);
    gathers run as one-hot matmuls on PE.
  - per-position score z = h_proj . score_w folds the same way into 4
    256-entry tables g_k.
  - multi-scale block means of z via symmetric block-average matrices
    A_bs (PE matmuls); softmax over the 4 block scales.
  - the L x L consensus attention exp(q_i . q_j) factorizes exactly
    (scores live on the 3-simplex; exp approximated by a cubic) as
    psi(q_i)^T M psi(q_j) with 20 monomial features -> attention
    collapses to W = sum_j psi_j [q_j|1]^T, out = psi M W. No L x L.
  - fused blockrepr x score mixing + 4x mean pool become per-tile
    [120,30] matmuls with runtime-weighted pooling matrices.

SPMD: one program, 8 in_maps. Each core sees a "local" block of 18
tiles x 120 positions (its half, conv-extended ids) plus the "remote"
block (other half) for the global score/attention sums; duplicate and
pad positions are zeroed via an uploaded psi mask.
"""

import sys

for _p in ("/opt/trn_rl_repo", "/opt/trn_rl_repo/concourse"):
    if _p not in sys.path:
        sys.path.insert(0, _p)

import numpy as np
import ml_dtypes

K = 4
BLOCKS = (1, 2, 3, 4)
DS = 4
DIM = 512
NTOK = 256
N = 4096
L = 4104
B = 4
NC = 8

TS = 120            # positions per tile (divisible by lcm(1,2,3,4) and DS)
NT_LOC = 18         # local tiles per core (2160 positions)
NT = 36             # local + remote tiles in score pipeline
BLK = TS * NT_LOC   # 2160 positions per block
IDS_LEN = BLK + K - 1   # 2163 ids per block (conv lookahead)
IDSP = 2176         # padded ids row length
NF = 20             # simplex monomial features (degree <= 3)
DSR = TS // DS      # 30 ds rows per tile
OUTR = NT_LOC * DSR  # 540 output rows per core

BF16 = ml_dtypes.bfloat16

_CACHE = {}


def _bf(a):
    return np.asarray(a, np.float32).astype(BF16)


def _poly_M():
    """Bilinear matrix M with psi(q)^T M psi(k) ~= exp(q.k) on the simplex."""
    xs = np.linspace(0.0, 1.0, 2001)
    V = np.vander(xs, 4, increasing=True)
    coef, *_ = np.linalg.lstsq(V, np.exp(xs), rcond=None)
    rng = np.random.default_rng(0)

    def samp(n):
        e = rng.exponential(size=(n, 4))
        return (e / e.sum(1, keepdims=True)).astype(np.float64)

    Q = samp(4000)
    Kk = samp(4000)
    PQ = _monomials(Q)
    PK = _monomials(Kk)
    S = Q @ Kk.T
    E = sum(c * (S ** m) for m, c in enumerate(coef))
    M = np.linalg.pinv(PQ) @ E @ np.linalg.pinv(PK).T
    return M.astype(np.float32)


def _monomials(q):
    q1, q2, q3 = q[..., 0], q[..., 1], q[..., 2]
    one = np.ones_like(q1)
    return np.stack(
        [one, q1, q2, q3,
         q1 * q1, q1 * q2, q1 * q3, q2 * q2, q2 * q3, q3 * q3,
         q1 * q1 * q1, q1 * q1 * q2, q1 * q1 * q3, q1 * q2 * q2,
         q1 * q2 * q3, q1 * q3 * q3, q2 * q2 * q2, q2 * q2 * q3,
         q2 * q3 * q3, q3 * q3 * q3],
        axis=-1,
    )


def _pool_mats():
    """A_bs [120,120] block-average (for scores); Pt_bs [120,120]
    block-broadcast-sum / bs (for fused reprs); P4 [120,30] ds-pool*0.25."""
    at = np.zeros((TS, 4, TS), np.float32)
    pt = np.zeros((TS, 4, TS), np.float32)
    for ki, bs in enumerate(BLOCKS):
        m = np.zeros((TS, TS), np.float32)
        for j in range(TS // bs):
            m[j * bs:(j + 1) * bs, j * bs:(j + 1) * bs] = 1.0 / bs
        at[:, ki, :] = m
        pt[:, ki, :] = m  # block-sum / bs (1/bs fused fold); k=1 -> identity
    p4 = np.zeros((TS, DSR), np.float32)
    for j in range(DSR):
        p4[j * DS:(j + 1) * DS, j] = 0.25
    return at, pt, p4


def _build_module():
    from concourse import bass, bacc, tile
    from concourse.bass import mybir

    f32 = mybir.dt.float32
    bf16 = mybir.dt.bfloat16
    nc = bacc.Bacc("TRN2", target_bir_lowering=False, debug=False,
                   num_devices=NC)

    def din(name, shape, dt=f32):
        return nc.dram_tensor(name, shape, dt, kind="ExternalInput")

    d_gt = din("gt", [128, 2, K, DIM], bf16)
    d_zt = din("ztab", [128, 2, K, 1], bf16)
    d_at = din("at", [TS, 4, TS], bf16)
    d_pt = din("pt", [TS, 4, TS], bf16)
    d_p4 = din("p4t", [TS, DSR], bf16)
    d_idt = din("idt", [TS, TS], f32)
    d_id1 = din("id1", [1, 1], f32)
    d_o128 = din("ones128", [1, 128], bf16)
    d_ocol = din("onescol", [TS, 1], f32)
    d_iota = din("iota", [128, 2], f32)
    d_mt = din("mt", [NF, NF], f32)
    d_ids = din("ids", [1, 2, IDSP], bf16)
    d_pm = din("pmask", [TS, NT], f32)
    d_out = nc.dram_tensor("out", [OUTR, DIM], f32, kind="ExternalOutput")

    EXP = mybir.ActivationFunctionType.Exp
    MUL = mybir.AluOpType.mult
    ADD = mybir.AluOpType.add
    ISEQ = mybir.AluOpType.is_equal
    X = mybir.AxisListType.X

    with tile.TileContext(nc) as tc:
        with tc.tile_pool(name="const", bufs=1) as cpool, \
             tc.tile_pool(name="sbuf", bufs=1) as spool, \
             tc.tile_pool(name="work", bufs=2) as wpool, \
             tc.tile_pool(name="pbig", bufs=2, space="PSUM") as pbig, \
             tc.tile_pool(name="pods", bufs=2, space="PSUM") as pods, \
             tc.tile_pool(name="poa", bufs=1, space="PSUM") as poa, \
             tc.tile_pool(name="psr", bufs=1, space="PSUM") as psr, \
             tc.tile_pool(name="pseq", bufs=2, space="PSUM") as pseq:

            def cload(dram, shape, dt, tag, eng=None):
                t = cpool.tile(shape, dt, tag=tag)
                (eng or nc.sync).dma_start(t[:], dram.ap()[:])
                return t

            ids = cload(d_ids, [1, 2, IDSP], bf16, "ids")
            iota = cload(d_iota, [128, 2], f32, "iota")
            o128 = cload(d_o128, [1, 128], bf16, "o128")
            ztab = cload(d_zt, [128, 2, K, 1], bf16, "ztab")
            id1 = cload(d_id1, [1, 1], f32, "id1")
            gt = cload(d_gt, [128, 2, K, DIM], bf16, "gt", nc.gpsimd)
            at = cload(d_at, [TS, 4, TS], bf16, "at", nc.gpsimd)
            pt = cload(d_pt, [TS, 4, TS], bf16, "pt", nc.gpsimd)
            p4t = cload(d_p4, [TS, DSR], bf16, "p4t")
            idt = cload(d_idt, [TS, TS], f32, "idt", nc.gpsimd)
            ocol = cload(d_ocol, [TS, 1], f32, "ocol")
            mt = cload(d_mt, [NF, NF], f32, "mt")
            pmask = cload(d_pm, [TS, NT], f32, "pmask")

            # persistent SBUF buffers
            oh = spool.tile([128, 2, 2, IDSP], bf16)     # onehot [vc, blk, pos]
            h_sb = spool.tile([TS, NT_LOC, DIM], bf16)   # local h_proj
            ztl = spool.tile([TS, NT], bf16)             # z per tile col
            zrow = spool.tile([1, 2, IDSP], f32)           # remote z row
            u_sb = spool.tile([TS, NT, 4], f32)
            den = spool.tile([TS, NT], f32)
            q_sb = spool.tile([TS, NT, 4], f32)
            psi = spool.tile([TS, NT, NF], f32)
            w_sb = spool.tile([NF, 4], f32)
            wq = spool.tile([NF, 4], f32)
            den2 = spool.tile([TS, NT_LOC], f32)
            fsum = spool.tile([TS, NT_LOC], f32)
            fp = spool.tile([TS, NT_LOC, 4], f32)
            F4 = spool.tile([TS, NT_LOC, 4, DSR], bf16)
            g2 = spool.tile([TS, NT_LOC, DSR], bf16)

            # ---- 1. one-hot build: broadcast ids down 128 partitions, compare to iota
            SL = [(0, 512), (512, 512), (1024, 512), (1536, 512), (2048, 128)]
            for blk in range(2):
                for (s0, w) in SL:
                    idb = pbig.tile([128, 512], f32, tag="a")
                    nc.tensor.matmul(idb[:, :w], lhsT=o128[:, :],
                                     rhs=ids[:, blk, s0:s0 + w],
                                     start=True, stop=True)
                    for vc in range(2):
                        nc.vector.tensor_scalar(
                            out=oh[:, vc, blk, s0:s0 + w], in0=idb[:, :w],
                            scalar1=iota[:, vc:vc + 1], scalar2=None, op0=ISEQ)

            # ---- 2. z rows (one-hot moving), PE-transpose into ztl columns
            SLZ = [(0, 512), (512, 512), (1024, 512), (1536, 512), (2048, 115)]
            for blk in range(2):
                for (s0, w) in SLZ:
                    zr_ps = pseq.tile([1, 512], f32, tag="s")
                    first = True
                    for k in range(K):
                        for vc in range(2):
                            nc.tensor.matmul(
                                zr_ps[:, :w], lhsT=ztab[:, vc, k, :],
                                rhs=oh[:, vc, blk, s0 + k: s0 + k + w],
                                start=first, stop=(k == K - 1 and vc == 1))
                            first = False
                    nc.vector.tensor_copy(zrow[:, blk, s0:s0 + w], zr_ps[:, :w])
            zz = pseq.tile([TS, NT], f32, tag="s")
            for blk in range(2):
                for j in range(NT_LOC):
                    nc.tensor.transpose(
                        zz[:, NT_LOC * blk + j: NT_LOC * blk + j + 1],
                        in_=zrow[:, blk, TS * j: TS * j + TS],
                        identity=id1[:, :])
            nc.vector.tensor_copy(ztl[:], zz[:])

            # ---- 3. block-scale scores (batched over all 36 tiles)
            sraw = psr.tile([TS, 4, NT], f32)
            for k in range(4):
                nc.tensor.matmul(sraw[:, k, :], lhsT=at[:, k, :], rhs=ztl[:],
                                 start=True, stop=True)

            # ---- 4a. first half of gcp (PE) — overlaps the softmax/psi DVE work
            def gcp_tile(t):
                h_ps0 = pbig.tile([128, DIM], f32, tag="a")
                h_ps = h_ps0[:TS, :]
                first = True
                for k in range(K):
                    for vc in range(2):
                        lhs = oh[:, vc, 0, TS * t + k: TS * t + k + TS]
                        nc.tensor.matmul(h_ps[:], lhsT=lhs, rhs=gt[:, vc, k, :],
                                         start=first,
                                         stop=(k == K - 1 and vc == 1))
                        first = False
                nc.scalar.copy(h_sb[:, t, :], h_ps[:])

            for t in range(0, 9):
                gcp_tile(t)

            # ---- 4b. softmax over scales + psi features (DVE/ACT, under gcp)
            nc.scalar.activation(u_sb[:].rearrange("p t k -> p k t"), sraw[:],
                                 EXP, bias=0.0, scale=1.0)
            nc.vector.tensor_reduce(den[:], u_sb[:], axis=X, op=ADD)
            nc.vector.reciprocal(den[:], den[:])
            nc.vector.tensor_tensor(
                q_sb[:], u_sb[:],
                den[:].unsqueeze(2).broadcast_to([TS, NT, 4]), op=MUL)

            nc.vector.memset(psi[:, :, 0:1], 1.0)
            nc.vector.tensor_copy(psi[:, :, 1:4], q_sb[:, :, 0:3])
            deg2 = [(4, 1, 1), (5, 1, 2), (6, 1, 3), (7, 2, 2), (8, 2, 3),
                    (9, 3, 3)]
            deg3 = [(10, 1, 4), (11, 1, 5), (12, 1, 6), (13, 1, 7), (14, 1, 8),
                    (15, 1, 9), (16, 2, 7), (17, 2, 8), (18, 2, 9), (19, 3, 9)]
            for (o, a, bb) in deg2 + deg3:
                nc.vector.tensor_tensor(psi[:, :, o:o + 1], psi[:, :, a:a + 1],
                                        psi[:, :, bb:bb + 1], op=MUL)
            nc.vector.tensor_tensor(
                psi[:], psi[:],
                pmask[:].unsqueeze(2).broadcast_to([TS, NT, NF]), op=MUL)
            vaug = spool.tile([TS, NT, 4], f32)
            nc.vector.tensor_copy(vaug[:, :, 0:3], q_sb[:, :, 0:3])
            nc.vector.memset(vaug[:, :, 3:4], 1.0)

            # ---- 5. W = sum_j psi_j [q_j(1:3) | 1]
            wps = pseq.tile([NF, 4], f32, tag="s")
            for t in range(NT):
                nc.tensor.matmul(wps[:], lhsT=psi[:, t, :], rhs=vaug[:, t, :],
                                 start=(t == 0), stop=(t == NT - 1))
            nc.vector.tensor_copy(w_sb[:], wps[:])
            wqp = pseq.tile([NF, 4], f32, tag="s")
            nc.tensor.matmul(wqp[:], lhsT=mt[:, :], rhs=w_sb[:],
                             start=True, stop=True)
            nc.vector.tensor_copy(wq[:], wqp[:])

            # ---- 6. out_aug = psi (M W) for local tiles
            oa = poa.tile([TS, NT_LOC * 4], f32)
            for t in range(NT_LOC):
                pT = pseq.tile([NF, TS], f32, tag="s")
                nc.tensor.transpose(pT[:], in_=psi[:, t, :], identity=idt[:, :])
                pTs = wpool.tile([NF, TS], f32, tag="pTs")
                nc.scalar.copy(pTs[:], pT[:])
                nc.tensor.matmul(oa[:, 4 * t: 4 * t + 4], lhsT=pTs[:],
                                 rhs=wq[:], start=True, stop=True)

            # ---- 7a. second half of gcp — overlaps the fscore/F DVE work
            for t in range(9, NT_LOC):
                gcp_tile(t)

            # ---- 7b. consensus scores f (f4 = 1 - f1 - f2 - f3)
            oa3 = oa[:].rearrange("p (t k) -> p t k", k=4)
            nc.vector.tensor_scalar_max(den2[:], oa3[:, :, 3], 1e-20)
            nc.vector.reciprocal(den2[:], den2[:])
            nc.vector.tensor_tensor(
                fp[:, :, 0:3], oa3[:, :, 0:3],
                den2[:].unsqueeze(2).broadcast_to([TS, NT_LOC, 3]), op=MUL)
            nc.vector.tensor_reduce(fsum[:], fp[:, :, 0:3], axis=X, op=ADD)
            nc.vector.tensor_scalar(out=fp[:, :, 3:4].rearrange("p t k -> p (t k)"),
                                    in0=fsum[:], scalar1=-1.0, scalar2=1.0,
                                    op0=MUL, op1=ADD)
            nc.vector.tensor_tensor(
                F4[:],
                fp[:].unsqueeze(3).broadcast_to([TS, NT_LOC, 4, DSR]),
                p4t[:].unsqueeze(1).unsqueeze(1).broadcast_to(
                    [TS, NT_LOC, 4, DSR]),
                op=MUL)

            # ---- 8. Gall = sum_k Pt_k @ F_k (k summed in PSUM), one copy out
            ga0 = pbig.tile([128, 510], f32, tag="a")
            ga = ga0[:TS, :]
            gb = pseq.tile([TS, DSR], f32, tag="s")
            for t in range(NT_LOC):
                out_ap = ga[:, DSR * t: DSR * t + DSR] if t < 17 else gb[:]
                for k in range(4):
                    nc.tensor.matmul(out_ap, lhsT=pt[:, k, :],
                                     rhs=F4[:, t, k, :],
                                     start=(k == 0), stop=(k == 3))
            nc.vector.tensor_copy(
                g2[:, 0:17, :], ga[:].rearrange("p (t c) -> p t c", c=DSR))
            nc.vector.tensor_copy(g2[:, 17, :], gb[:])

            # ---- 9. fused mixing + 4x downsample pool, DMA out
            for t in range(NT_LOC):
                ods = pods.tile([DSR, DIM], f32, tag="ods")
                nc.tensor.matmul(ods[:], lhsT=g2[:, t, :], rhs=h_sb[:, t, :],
                                 start=True, stop=True)
                osb = wpool.tile([DSR, DIM], f32, tag="osb")
                nc.scalar.copy(osb[:], ods[:])
                nc.sync.dma_start(d_out.ap()[DSR * t: DSR * t + DSR, :], osb[:])

    nc.compile()
    return nc


def _host_prep(emb, conv_w, conv_b, proj_w, proj_b, score_w, score_b):
    G = np.stack([(emb * conv_w[:, k][None, :]) @ proj_w.T for k in range(K)])
    C = conv_b @ proj_w.T + proj_b
    G[0] += C
    g = G @ score_w                      # [4, 256]
    gt = np.zeros((128, 2, K, DIM), np.float32)
    zt = np.zeros((128, 2, K, 1), np.float32)
    for vc in range(2):
        for k in range(K):
            gt[:, vc, k, :] = G[k][128 * vc:128 * vc + 128]
            zt[:, vc, k, 0] = g[k][128 * vc:128 * vc + 128]
    at, pt, p4 = _pool_mats()
    M = _poly_M()
    iota = np.stack([np.arange(128, dtype=np.float32),
                     np.arange(128, 256, dtype=np.float32)], axis=1)
    consts = {
        "gt": _bf(gt), "ztab": _bf(zt), "at": _bf(at), "pt": _bf(pt),
        "p4t": _bf(p4), "idt": np.eye(TS, dtype=np.float32),
        "id1": np.ones((1, 1), np.float32), "ones128": _bf(np.ones((1, 128))),
        "onescol": np.ones((TS, 1), np.float32), "iota": iota,
        "mt": np.ascontiguousarray(M.T),
    }
    return consts, np.float32(score_b)


def _core_inputs(x_row, hi):
    """ids [1,2,IDSP] bf16 and pmask [TS,NT] f32 for core half hi."""
    idsp = np.full(L + K - 1, -1.0, np.float32)
    idsp[:N] = x_row.astype(np.float32)

    def block(start):
        out = np.full(IDSP, -1.0, np.float32)
        lo = start
        hhi = min(start + IDS_LEN, L + K - 1)
        out[:hhi - lo] = idsp[lo:hhi]
        return out

    o_loc = 0 if hi == 0 else L - BLK          # 0 or 1944? no: 2040
    o_loc = 0 if hi == 0 else 2040
    o_rem = 2040 if hi == 0 else 0
    ids = np.stack([block(o_loc), block(o_rem)])[None]  # [1,2,IDSP]

    pm = np.zeros((TS, NT), np.float32)
    for j in range(NT):
        base = (o_loc if j < NT_LOC else o_rem) + TS * (j % NT_LOC)
        gpos = base + np.arange(TS)
        valid = gpos < L
        if hi == 0 and j == NT_LOC:          # remote tile at 2040 duplicates local t=17
            valid &= False
        if hi == 1 and j == NT - 1:          # remote tile at 2040 duplicates local t=0
            valid &= False
        pm[:, j] = valid.astype(np.float32)
    return _bf(ids), pm


def kernel(x, emb, conv_w, conv_b, proj_w, proj_b, score_w, score_b):
    from concourse import bass_utils

    x = np.asarray(x)
    emb = np.asarray(emb, np.float32)
    conv_w = np.asarray(conv_w, np.float32)
    conv_b = np.asarray(conv_b, np.float32)
    proj_w = np.asarray(proj_w, np.float32)
    proj_b = np.asarray(proj_b, np.float32)
    score_w = np.asarray(score_w, np.float32)
    score_b = np.float32(np.asarray(score_b))

    if "nc" not in _CACHE:
        _CACHE["nc"] = _build_module()
    nc = _CACHE["nc"]

    consts, sb = _host_prep(emb, conv_w, conv_b, proj_w, proj_b,
                            score_w, score_b)
    # score_b is folded as exp bias -> bake into ztab? No: it is a bias on
    # scores_raw. We add it on host into the A_bs result via ztab offset:
    # scores_raw = A_bs z + score_b. Instead fold into z: z' = z + score_b
    # would shift all scales equally -> softmax invariant? No: A_bs averages
    # z, so adding score_b to every z entry adds score_b to every score --
    # softmax over k is invariant to a common shift. pad positions get
    # masked anyway. So score_b can be DROPPED entirely.
    del sb

    in_maps = []
    for c in range(NC):
        bi, hi = divmod(c, 2)
        ids, pm = _core_inputs(x[bi], hi)
        m = dict(consts)
        m["ids"] = ids
        m["pmask"] = pm
        in_maps.append(m)

    res = bass_utils.run_bass_kernel_spmd(
        nc, in_maps, core_ids=list(range(NC)), trace=_CACHE.get("trace", False))
    _CACHE["last_exec_ns"] = res.exec_time_ns

    out = np.empty((B, N // DS, DIM), np.float32)
    for c in range(NC):
        bi, hi = divmod(c, 2)
        r = res.results[c]["out"]
        if hi == 0:
            out[bi, 0:540] = r[0:540]
        else:
            out[bi, 540:1024] = r[30:514]
    return out


# revision 15
# speedup vs baseline: 1.8736x; 1.2000x over previous
"""GBST (segment_reduce) Trainium2 Bass kernel — nn_GBST_26061861552188.

kernel(**inputs) takes FULL unsharded inputs, returns FULL output
[4, 1024, 512] f32. 8 NeuronCores, data-parallel over (batch x seq-half).

Math (validated vs reference in numpy, rel err 2.8e-3 with bf16 tables):
  - emb gather + depthwise conv(K=4) + 1x1 proj fold into 4 per-shift
    lookup tables G_k = diag(conv_w[:,k]) emb @ proj_w.T (+C into G_0);
    gathers run as one-hot matmuls on PE.
  - per-position score z = h_proj . score_w folds the same way into 4
    256-entry tables g_k.
  - multi-scale block means of z via symmetric block-average matrices
    A_bs (PE matmuls); softmax over the 4 block scales.
  - the L x L consensus attention exp(q_i . q_j) factorizes exactly
    (scores live on the 3-simplex; exp approximated by a cubic) as
    psi(q_i)^T M psi(q_j) with 20 monomial features -> attention
    collapses to W = sum_j psi_j [q_j|1]^T, out = psi M W. No L x L.
  - fused blockrepr x score mixing + 4x mean pool become per-tile
    [120,30] matmuls with runtime-weighted pooling matrices.

SPMD: one program, 8 in_maps. Each core sees a "local" block of 18
tiles x 120 positions (its half, conv-extended ids) plus the "remote"
block (other half) for the global score/attention sums; duplicate and
pad positions are zeroed via an uploaded psi mask.
"""

import sys

for _p in ("/opt/trn_rl_repo", "/opt/trn_rl_repo/concourse"):
    if _p not in sys.path:
        sys.path.insert(0, _p)

import numpy as np
import ml_dtypes

K = 4
BLOCKS = (1, 2, 3, 4)
DS = 4
DIM = 512
NTOK = 256
N = 4096
L = 4104
B = 4
NC = 8

TS = 120            # positions per tile (divisible by lcm(1,2,3,4) and DS)
NT_LOC = 18         # local tiles per core (2160 positions)
NT = 36             # local + remote tiles in score pipeline
BLK = TS * NT_LOC   # 2160 positions per block
IDS_LEN = BLK + K - 1   # 2163 ids per block (conv lookahead)
IDSP = 2176         # padded ids row length
NF = 20             # simplex monomial features (degree <= 3)
DSR = TS // DS      # 30 ds rows per tile
OUTR = NT_LOC * DSR  # 540 output rows per core

BF16 = ml_dtypes.bfloat16

_CACHE = {}


def _bf(a):
    return np.asarray(a, np.float32).astype(BF16)


def _poly_M():
    """Bilinear matrix M with psi(q)^T M psi(k) ~= exp(q.k) on the simplex."""
    xs = np.linspace(0.0, 1.0, 2001)
    V = np.vander(xs, 4, increasing=True)
    coef, *_ = np.linalg.lstsq(V, np.exp(xs), rcond=None)
    rng = np.random.default_rng(0)

    def samp(n):
        e = rng.exponential(size=(n, 4))
        return (e / e.sum(1, keepdims=True)).astype(np.float64)

    Q = samp(4000)
    Kk = samp(4000)
    PQ = _monomials(Q)
    PK = _monomials(Kk)
    S = Q @ Kk.T
    E = sum(c * (S ** m) for m, c in enumerate(coef))
    M = np.linalg.pinv(PQ) @ E @ np.linalg.pinv(PK).T
    return M.astype(np.float32)


def _monomials(q):
    q1, q2, q3 = q[..., 0], q[..., 1], q[..., 2]
    one = np.ones_like(q1)
    return np.stack(
        [one, q1, q2, q3,
         q1 * q1, q1 * q2, q1 * q3, q2 * q2, q2 * q3, q3 * q3,
         q1 * q1 * q1, q1 * q1 * q2, q1 * q1 * q3, q1 * q2 * q2,
         q1 * q2 * q3, q1 * q3 * q3, q2 * q2 * q2, q2 * q2 * q3,
         q2 * q3 * q3, q3 * q3 * q3],
        axis=-1,
    )


def _pool_mats():
    """A_bs [120,120] block-average (for scores); Pt_bs [120,120]
    block-broadcast-sum / bs (for fused reprs); P4 [120,30] ds-pool*0.25."""
    at = np.zeros((TS, 4, TS), np.float32)
    pt = np.zeros((TS, 4, TS), np.float32)
    for ki, bs in enumerate(BLOCKS):
        m = np.zeros((TS, TS), np.float32)
        for j in range(TS // bs):
            m[j * bs:(j + 1) * bs, j * bs:(j + 1) * bs] = 1.0 / bs
        at[:, ki, :] = m
        pt[:, ki, :] = m  # block-sum / bs (1/bs fused fold); k=1 -> identity
    p4 = np.zeros((TS, DSR), np.float32)
    for j in range(DSR):
        p4[j * DS:(j + 1) * DS, j] = 0.25
    return at, pt, p4


def _build_module():
    from concourse import bass, bacc, tile
    from concourse.bass import mybir

    f32 = mybir.dt.float32
    bf16 = mybir.dt.bfloat16
    nc = bacc.Bacc("TRN2", target_bir_lowering=False, debug=False,
                   num_devices=NC)

    def din(name, shape, dt=f32):
        return nc.dram_tensor(name, shape, dt, kind="ExternalInput")

    d_gt = din("gt", [128, 2, K, DIM], bf16)
    d_zt = din("ztab", [128, 2, K, 1], bf16)
    d_at = din("at", [TS, 4, TS], bf16)
    d_pt = din("pt", [TS, 4, TS], bf16)
    d_p4 = din("p4t", [TS, DSR], bf16)
    d_idt = din("idt", [TS, TS], f32)
    d_id1 = din("id1", [1, 1], f32)
    d_o128 = din("ones128", [1, 128], bf16)
    d_iota = din("iota", [128, 2], f32)
    d_mt = din("mt", [NF, NF], f32)
    d_ids = din("ids", [1, 2, IDSP], bf16)
    d_pm = din("pmask", [TS, NT], f32)
    d_out = nc.dram_tensor("out", [OUTR, DIM], f32, kind="ExternalOutput")

    EXP = mybir.ActivationFunctionType.Exp
    MUL = mybir.AluOpType.mult
    ADD = mybir.AluOpType.add
    ISEQ = mybir.AluOpType.is_equal
    X = mybir.AxisListType.X

    with tile.TileContext(nc) as tc:
        with tc.tile_pool(name="const", bufs=1) as cpool, \
             tc.tile_pool(name="sbuf", bufs=1) as spool, \
             tc.tile_pool(name="work", bufs=2) as wpool, \
             tc.tile_pool(name="pbig", bufs=2, space="PSUM") as pbig, \
             tc.tile_pool(name="pods", bufs=3, space="PSUM") as pods, \
             tc.tile_pool(name="psr", bufs=1, space="PSUM") as psr, \
             tc.tile_pool(name="pseq", bufs=2, space="PSUM") as pseq:

            def cload(dram, shape, dt, tag, eng=None):
                t = cpool.tile(shape, dt, tag=tag)
                (eng or nc.sync).dma_start(t[:], dram.ap()[:])
                return t

            ids = cload(d_ids, [1, 2, IDSP], bf16, "ids")
            iota = cload(d_iota, [128, 2], f32, "iota")
            o128 = cload(d_o128, [1, 128], bf16, "o128")
            ztab = cload(d_zt, [128, 2, K, 1], bf16, "ztab", nc.scalar)
            id1 = cload(d_id1, [1, 1], f32, "id1", nc.scalar)
            gt = cload(d_gt, [128, 2, K, DIM], bf16, "gt", nc.gpsimd)
            at = cload(d_at, [TS, 4, TS], bf16, "at", nc.gpsimd)
            pt = cload(d_pt, [TS, 4, TS], bf16, "pt", nc.scalar)
            p4t = cload(d_p4, [TS, DSR], bf16, "p4t", nc.scalar)
            idt = cload(d_idt, [TS, TS], f32, "idt", nc.gpsimd)
            mt = cload(d_mt, [NF, NF], f32, "mt", nc.scalar)
            pmask = cload(d_pm, [TS, NT], f32, "pmask", nc.scalar)

            # persistent SBUF buffers
            oh = spool.tile([128, 2, 2, IDSP], bf16)     # onehot [vc, blk, pos]
            h_sb = spool.tile([TS, NT_LOC, DIM], bf16)   # local h_proj
            ztl = spool.tile([TS, NT], bf16)             # z per tile col
            zrow = spool.tile([1, 2, IDSP], f32)           # remote z row
            u_sb = spool.tile([TS, NT, 4], f32)
            den = spool.tile([TS, NT], f32)
            q_sb = spool.tile([TS, NT, 4], f32)
            psi = spool.tile([TS, NT, NF], f32)
            w_sb = spool.tile([NF, 4], f32)
            wq = spool.tile([NF, 4], f32)
            den2 = spool.tile([TS, NT_LOC], f32)
            fsum = spool.tile([TS, NT_LOC], f32)
            fp = spool.tile([TS, NT_LOC, 4], f32)
            F4 = spool.tile([TS, NT_LOC, 4, DSR], bf16)
            g2 = spool.tile([TS, NT_LOC, DSR], bf16)

            # ---- 1. one-hot build: broadcast ids down 128 partitions, compare to iota
            SL = [(0, 512), (512, 512), (1024, 512), (1536, 512), (2048, 128)]
            for blk in range(2):
                for (s0, w) in SL:
                    idb = pbig.tile([128, 512], f32, tag="a")
                    nc.tensor.matmul(idb[:, :w], lhsT=o128[:, :],
                                     rhs=ids[:, blk, s0:s0 + w],
                                     start=True, stop=True)
                    for vc in range(2):
                        nc.vector.tensor_scalar(
                            out=oh[:, vc, blk, s0:s0 + w], in0=idb[:, :w],
                            scalar1=iota[:, vc:vc + 1], scalar2=None, op0=ISEQ)

            # ---- 2. z rows (one-hot moving), PE-transpose into ztl columns
            SLZ = [(0, 512), (512, 512), (1024, 512), (1536, 512), (2048, 115)]
            for blk in range(2):
                for (s0, w) in SLZ:
                    zr_ps = pseq.tile([1, 512], f32, tag="s")
                    first = True
                    for k in range(K):
                        for vc in range(2):
                            nc.tensor.matmul(
                                zr_ps[:, :w], lhsT=ztab[:, vc, k, :],
                                rhs=oh[:, vc, blk, s0 + k: s0 + k + w],
                                start=first, stop=(k == K - 1 and vc == 1))
                            first = False
                    nc.vector.tensor_copy(zrow[:, blk, s0:s0 + w], zr_ps[:, :w])
            zz = pseq.tile([TS, NT], f32, tag="s")
            for blk in range(2):
                for j in range(NT_LOC):
                    nc.tensor.transpose(
                        zz[:, NT_LOC * blk + j: NT_LOC * blk + j + 1],
                        in_=zrow[:, blk, TS * j: TS * j + TS],
                        identity=id1[:, :])
            nc.vector.tensor_copy(ztl[:], zz[:])

            # ---- 3. block-scale scores (batched over all 36 tiles)
            sraw = psr.tile([TS, 4, NT], f32, tag="sr")
            for k in range(4):
                nc.tensor.matmul(sraw[:, k, :], lhsT=at[:, k, :], rhs=ztl[:],
                                 start=True, stop=True)

            # ---- 4a. first half of gcp (PE) — overlaps the softmax/psi DVE work
            def gcp_tile(t):
                h_ps0 = pbig.tile([128, DIM], f32, tag="a")
                h_ps = h_ps0[:TS, :]
                first = True
                for k in range(K):
                    for vc in range(2):
                        lhs = oh[:, vc, 0, TS * t + k: TS * t + k + TS]
                        nc.tensor.matmul(h_ps[:], lhsT=lhs, rhs=gt[:, vc, k, :],
                                         start=first,
                                         stop=(k == K - 1 and vc == 1))
                        first = False
                nc.scalar.copy(h_sb[:, t, :], h_ps[:])

            for t in range(0, 9):
                gcp_tile(t)

            # ---- 4b. softmax over scales + psi features (DVE/ACT, under gcp)
            nc.scalar.activation(u_sb[:].rearrange("p t k -> p k t"), sraw[:],
                                 EXP, bias=0.0, scale=1.0)
            nc.vector.tensor_reduce(den[:], u_sb[:], axis=X, op=ADD)
            nc.vector.reciprocal(den[:], den[:])
            nc.vector.tensor_tensor(
                q_sb[:], u_sb[:],
                den[:].unsqueeze(2).broadcast_to([TS, NT, 4]), op=MUL)

            nc.vector.memset(psi[:, :, 0:1], 1.0)
            nc.vector.tensor_copy(psi[:, :, 1:4], q_sb[:, :, 0:3])
            deg2 = [(4, 1, 1), (5, 1, 2), (6, 1, 3), (7, 2, 2), (8, 2, 3),
                    (9, 3, 3)]
            deg3 = [(10, 1, 4), (11, 1, 5), (12, 1, 6), (13, 1, 7), (14, 1, 8),
                    (15, 1, 9), (16, 2, 7), (17, 2, 8), (18, 2, 9), (19, 3, 9)]
            for (o, a, bb) in deg2 + deg3:
                nc.vector.tensor_tensor(psi[:, :, o:o + 1], psi[:, :, a:a + 1],
                                        psi[:, :, bb:bb + 1], op=MUL)
            nc.vector.tensor_tensor(
                psi[:], psi[:],
                pmask[:].unsqueeze(2).broadcast_to([TS, NT, NF]), op=MUL)
            vaug = spool.tile([TS, NT, 4], f32)
            nc.vector.tensor_copy(vaug[:, :, 0:3], q_sb[:, :, 0:3])
            nc.vector.memset(vaug[:, :, 3:4], 1.0)

            # ---- 5. W = sum_j psi_j [q_j(1:3) | 1]
            wps = pseq.tile([NF, 4], f32, tag="s")
            for t in range(NT):
                nc.tensor.matmul(wps[:], lhsT=psi[:, t, :], rhs=vaug[:, t, :],
                                 start=(t == 0), stop=(t == NT - 1))
            nc.vector.tensor_copy(w_sb[:], wps[:])
            wqp = pseq.tile([NF, 4], f32, tag="s")
            nc.tensor.matmul(wqp[:], lhsT=mt[:, :], rhs=w_sb[:],
                             start=True, stop=True)
            nc.vector.tensor_copy(wq[:], wqp[:])

            # ---- 6. out_aug = psi (M W) for local tiles
            oa = psr.tile([TS, NT_LOC * 4], f32, tag="sr")
            for t in range(NT_LOC):
                pT = pseq.tile([NF, TS], f32, tag="s")
                nc.tensor.transpose(pT[:], in_=psi[:, t, :], identity=idt[:, :])
                pTs = wpool.tile([NF, TS], f32, tag="pTs")
                nc.scalar.copy(pTs[:], pT[:])
                nc.tensor.matmul(oa[:, 4 * t: 4 * t + 4], lhsT=pTs[:],
                                 rhs=wq[:], start=True, stop=True)

            # ---- 7a. second half of gcp — overlaps the fscore/F DVE work
            for t in range(9, NT_LOC):
                gcp_tile(t)

            # ---- 7b. consensus scores f (f4 = 1 - f1 - f2 - f3)
            oa3 = oa[:].rearrange("p (t k) -> p t k", k=4)
            nc.vector.tensor_scalar_max(den2[:], oa3[:, :, 3], 1e-20)
            nc.vector.reciprocal(den2[:], den2[:])
            nc.vector.tensor_tensor(
                fp[:, :, 0:3], oa3[:, :, 0:3],
                den2[:].unsqueeze(2).broadcast_to([TS, NT_LOC, 3]), op=MUL)
            nc.vector.tensor_reduce(fsum[:], fp[:, :, 0:3], axis=X, op=ADD)
            nc.vector.tensor_scalar(out=fp[:, :, 3:4].rearrange("p t k -> p (t k)"),
                                    in0=fsum[:], scalar1=-1.0, scalar2=1.0,
                                    op0=MUL, op1=ADD)
            nc.vector.tensor_tensor(
                F4[:],
                fp[:].unsqueeze(3).broadcast_to([TS, NT_LOC, 4, DSR]),
                p4t[:].unsqueeze(1).unsqueeze(1).broadcast_to(
                    [TS, NT_LOC, 4, DSR]),
                op=MUL)

            # ---- 8. Gall = sum_k Pt_k @ F_k; tiles ride the free dim,
            # one stationary per k, k accumulated in PSUM
            ga0 = pbig.tile([128, 510], f32, tag="a")
            ga = ga0[:TS, :]
            gb = pseq.tile([TS, DSR], f32, tag="s")
            for k in range(4):
                nc.tensor.matmul(ga[:], lhsT=pt[:, k, :],
                                 rhs=F4[:, 0:17, k, :],
                                 start=(k == 0), stop=(k == 3))
                nc.tensor.matmul(gb[:], lhsT=pt[:, k, :],
                                 rhs=F4[:, 17, k, :],
                                 start=(k == 0), stop=(k == 3))
            nc.vector.tensor_copy(
                g2[:, 0:17, :], ga[:].rearrange("p (t c) -> p t c", c=DSR))
            nc.vector.tensor_copy(g2[:, 17, :], gb[:])

            # ---- 9. fused mixing + 4x downsample pool, grouped DMA out
            osb = spool.tile([DSR, NT_LOC, DIM], f32)
            outv = d_out.ap().rearrange("(t p) d -> p t d", p=DSR)
            groups = [(0, 5, nc.sync), (5, 10, nc.gpsimd), (10, 14, nc.scalar),
                      (14, 18, nc.sync)]
            for (g0, g1, eng) in groups:
                for t in range(g0, g1):
                    ods = pods.tile([DSR, DIM], f32, tag="ods")
                    nc.tensor.matmul(ods[:], lhsT=g2[:, t, :],
                                     rhs=h_sb[:, t, :], start=True, stop=True)
                    if t % 2 == 0:
                        nc.scalar.copy(osb[:, t, :], ods[:])
                    else:
                        nc.vector.tensor_copy(osb[:, t, :], ods[:])
                eng.dma_start(outv[:, g0:g1, :], osb[:, g0:g1, :])

    nc.compile()
    return nc


def _host_prep(emb, conv_w, conv_b, proj_w, proj_b, score_w, score_b):
    G = np.stack([(emb * conv_w[:, k][None, :]) @ proj_w.T for k in range(K)])
    C = conv_b @ proj_w.T + proj_b
    G[0] += C
    g = G @ score_w                      # [4, 256]
    gt = np.zeros((128, 2, K, DIM), np.float32)
    zt = np.zeros((128, 2, K, 1), np.float32)
    for vc in range(2):
        for k in range(K):
            gt[:, vc, k, :] = G[k][128 * vc:128 * vc + 128]
            zt[:, vc, k, 0] = g[k][128 * vc:128 * vc + 128]
    at, pt, p4 = _pool_mats()
    M = _poly_M()
    iota = np.stack([np.arange(128, dtype=np.float32),
                     np.arange(128, 256, dtype=np.float32)], axis=1)
    consts = {
        "gt": _bf(gt), "ztab": _bf(zt), "at": _bf(at), "pt": _bf(pt),
        "p4t": _bf(p4), "idt": np.eye(TS, dtype=np.float32),
        "id1": np.ones((1, 1), np.float32), "ones128": _bf(np.ones((1, 128))),
        "iota": iota,
        "mt": np.ascontiguousarray(M.T),
    }
    return consts, np.float32(score_b)


def _core_inputs(x_row, hi):
    """ids [1,2,IDSP] bf16 and pmask [TS,NT] f32 for core half hi."""
    idsp = np.full(L + K - 1, -1.0, np.float32)
    idsp[:N] = x_row.astype(np.float32)

    def block(start):
        out = np.full(IDSP, -1.0, np.float32)
        lo = start
        hhi = min(start + IDS_LEN, L + K - 1)
        out[:hhi - lo] = idsp[lo:hhi]
        return out

    o_loc = 0 if hi == 0 else L - BLK          # 0 or 1944? no: 2040
    o_loc = 0 if hi == 0 else 2040
    o_rem = 2040 if hi == 0 else 0
    ids = np.stack([block(o_loc), block(o_rem)])[None]  # [1,2,IDSP]

    pm = np.zeros((TS, NT), np.float32)
    for j in range(NT):
        base = (o_loc if j < NT_LOC else o_rem) + TS * (j % NT_LOC)
        gpos = base + np.arange(TS)
        valid = gpos < L
        if hi == 0 and j == NT_LOC:          # remote tile at 2040 duplicates local t=17
            valid &= False
        if hi == 1 and j == NT - 1:          # remote tile at 2040 duplicates local t=0
            valid &= False
        pm[:, j] = valid.astype(np.float32)
    return _bf(ids), pm


def kernel(x, emb, conv_w, conv_b, proj_w, proj_b, score_w, score_b):
    from concourse import bass_utils

    x = np.asarray(x)
    emb = np.asarray(emb, np.float32)
    conv_w = np.asarray(conv_w, np.float32)
    conv_b = np.asarray(conv_b, np.float32)
    proj_w = np.asarray(proj_w, np.float32)
    proj_b = np.asarray(proj_b, np.float32)
    score_w = np.asarray(score_w, np.float32)
    score_b = np.float32(np.asarray(score_b))

    if "nc" not in _CACHE:
        _CACHE["nc"] = _build_module()
    nc = _CACHE["nc"]

    consts, sb = _host_prep(emb, conv_w, conv_b, proj_w, proj_b,
                            score_w, score_b)
    # score_b is folded as exp bias -> bake into ztab? No: it is a bias on
    # scores_raw. We add it on host into the A_bs result via ztab offset:
    # scores_raw = A_bs z + score_b. Instead fold into z: z' = z + score_b
    # would shift all scales equally -> softmax invariant? No: A_bs averages
    # z, so adding score_b to every z entry adds score_b to every score --
    # softmax over k is invariant to a common shift. pad positions get
    # masked anyway. So score_b can be DROPPED entirely.
    del sb

    in_maps = []
    for c in range(NC):
        bi, hi = divmod(c, 2)
        ids, pm = _core_inputs(x[bi], hi)
        m = dict(consts)
        m["ids"] = ids
        m["pmask"] = pm
        in_maps.append(m)

    res = bass_utils.run_bass_kernel_spmd(
        nc, in_maps, core_ids=list(range(NC)), trace=_CACHE.get("trace", False))
    _CACHE["last_exec_ns"] = res.exec_time_ns

    out = np.empty((B, N // DS, DIM), np.float32)
    for c in range(NC):
        bi, hi = divmod(c, 2)
        r = res.results[c]["out"]
        if hi == 0:
            out[bi, 0:540] = r[0:540]
        else:
            out[bi, 540:1024] = r[30:514]
    return out


# revision 19
# speedup vs baseline: 1.9072x; 1.0180x over previous
"""GBST (segment_reduce) Trainium2 Bass kernel — nn_GBST_26061861552188.

kernel(**inputs) takes FULL unsharded inputs, returns FULL output
[4, 1024, 512] f32. 8 NeuronCores, data-parallel over (batch x seq-half).

Math (validated vs reference in numpy, rel err 2.8e-3 with bf16 tables):
  - emb gather + depthwise conv(K=4) + 1x1 proj fold into 4 per-shift
    lookup tables G_k = diag(conv_w[:,k]) emb @ proj_w.T (+C into G_0);
    gathers run as one-hot matmuls on PE.
  - per-position score z = h_proj . score_w folds the same way into 4
    256-entry tables g_k.
  - multi-scale block means of z via symmetric block-average matrices
    A_bs (PE matmuls); softmax over the 4 block scales.
  - the L x L consensus attention exp(q_i . q_j) factorizes exactly
    (scores live on the 3-simplex; exp approximated by a cubic) as
    psi(q_i)^T M psi(q_j) with 20 monomial features -> attention
    collapses to W = sum_j psi_j [q_j|1]^T, out = psi M W. No L x L.
  - fused blockrepr x score mixing + 4x mean pool become per-tile
    [120,30] matmuls with runtime-weighted pooling matrices.

SPMD: one program, 8 in_maps. Each core sees a "local" block of 18
tiles x 120 positions (its half, conv-extended ids) plus the "remote"
block (other half) for the global score/attention sums; duplicate and
pad positions are zeroed via an uploaded psi mask.
"""

import sys

for _p in ("/opt/trn_rl_repo", "/opt/trn_rl_repo/concourse"):
    if _p not in sys.path:
        sys.path.insert(0, _p)

import numpy as np
import ml_dtypes

K = 4
BLOCKS = (1, 2, 3, 4)
DS = 4
DIM = 512
NTOK = 256
N = 4096
L = 4104
B = 4
NC = 8

TS = 120            # positions per tile (divisible by lcm(1,2,3,4) and DS)
NT_LOC = 18         # local tiles per core (2160 positions)
NT = 36             # local + remote tiles in score pipeline
BLK = TS * NT_LOC   # 2160 positions per block
IDS_LEN = BLK + K - 1   # 2163 ids per block (conv lookahead)
IDSP = 2176         # padded ids row length
NF = 20             # simplex monomial features (degree <= 3)
DSR = TS // DS      # 30 ds rows per tile
OUTR = NT_LOC * DSR  # 540 output rows per core

BF16 = ml_dtypes.bfloat16

_CACHE = {}


def _bf(a):
    return np.asarray(a, np.float32).astype(BF16)


def _poly_M():
    """Bilinear matrix M with psi(q)^T M psi(k) ~= exp(q.k) on the simplex."""
    xs = np.linspace(0.0, 1.0, 2001)
    V = np.vander(xs, 4, increasing=True)
    coef, *_ = np.linalg.lstsq(V, np.exp(xs), rcond=None)
    rng = np.random.default_rng(0)

    def samp(n):
        e = rng.exponential(size=(n, 4))
        return (e / e.sum(1, keepdims=True)).astype(np.float64)

    Q = samp(4000)
    Kk = samp(4000)
    PQ = _monomials(Q)
    PK = _monomials(Kk)
    S = Q @ Kk.T
    E = sum(c * (S ** m) for m, c in enumerate(coef))
    M = np.linalg.pinv(PQ) @ E @ np.linalg.pinv(PK).T
    return M.astype(np.float32)


def _monomials(q):
    q1, q2, q3 = q[..., 0], q[..., 1], q[..., 2]
    one = np.ones_like(q1)
    return np.stack(
        [one, q1, q2, q3,
         q1 * q1, q1 * q2, q1 * q3, q2 * q2, q2 * q3, q3 * q3,
         q1 * q1 * q1, q1 * q1 * q2, q1 * q1 * q3, q1 * q2 * q2,
         q1 * q2 * q3, q1 * q3 * q3, q2 * q2 * q2, q2 * q2 * q3,
         q2 * q3 * q3, q3 * q3 * q3],
        axis=-1,
    )


def _pool_mats():
    """A_bs [120,120] block-average (for scores); Pt_bs [120,120]
    block-broadcast-sum / bs (for fused reprs); P4 [120,30] ds-pool*0.25."""
    at = np.zeros((TS, 4, TS), np.float32)
    pt = np.zeros((TS, 4, TS), np.float32)
    for ki, bs in enumerate(BLOCKS):
        m = np.zeros((TS, TS), np.float32)
        for j in range(TS // bs):
            m[j * bs:(j + 1) * bs, j * bs:(j + 1) * bs] = 1.0 / bs
        at[:, ki, :] = m
        pt[:, ki, :] = m  # block-sum / bs (1/bs fused fold); k=1 -> identity
    p4 = np.zeros((TS, DSR), np.float32)
    for j in range(DSR):
        p4[j * DS:(j + 1) * DS, j] = 0.25
    return at, pt, p4


def _build_module():
    from concourse import bass, bacc, tile
    from concourse.bass import mybir

    f32 = mybir.dt.float32
    bf16 = mybir.dt.bfloat16
    nc = bacc.Bacc("TRN2", target_bir_lowering=False, debug=False,
                   num_devices=NC)

    def din(name, shape, dt=f32):
        return nc.dram_tensor(name, shape, dt, kind="ExternalInput")

    d_gt = din("gt", [128, 2, K, DIM], bf16)
    d_zt = din("ztab", [128, 2, K, 1], bf16)
    d_at = din("at", [TS, 4, TS], bf16)
    d_pt = din("pt", [TS, 4, TS], bf16)
    d_p4 = din("p4t", [TS, DSR], bf16)
    d_idt = din("idt", [TS, TS], f32)
    d_id1 = din("id1", [1, 1], f32)
    d_o128 = din("ones128", [1, 128], bf16)
    d_iota = din("iota", [128, 2], f32)
    d_mt = din("mt", [NF, NF], f32)
    d_ids = din("ids", [1, 2, IDSP], bf16)
    d_pm = din("pmask", [TS, NT], f32)
    d_out = nc.dram_tensor("out", [OUTR, DIM], f32, kind="ExternalOutput")

    EXP = mybir.ActivationFunctionType.Exp
    MUL = mybir.AluOpType.mult
    ADD = mybir.AluOpType.add
    ISEQ = mybir.AluOpType.is_equal
    X = mybir.AxisListType.X

    with tile.TileContext(nc) as tc:
        with tc.tile_pool(name="const", bufs=1) as cpool, \
             tc.tile_pool(name="sbuf", bufs=1) as spool, \
             tc.tile_pool(name="work", bufs=2) as wpool, \
             tc.tile_pool(name="dram", bufs=1, space="DRAM") as dpool, \
             tc.tile_pool(name="pbig", bufs=2, space="PSUM") as pbig, \
             tc.tile_pool(name="pods", bufs=3, space="PSUM") as pods, \
             tc.tile_pool(name="psr", bufs=1, space="PSUM") as psr, \
             tc.tile_pool(name="pseq", bufs=2, space="PSUM") as pseq:

            def cload(dram, shape, dt, tag, eng=None):
                t = cpool.tile(shape, dt, tag=tag)
                (eng or nc.sync).dma_start(t[:], dram.ap()[:])
                return t

            ids = cload(d_ids, [1, 2, IDSP], bf16, "ids")
            iota = cload(d_iota, [128, 2], f32, "iota", nc.gpsimd)
            o128 = cload(d_o128, [1, 128], bf16, "o128", nc.scalar)
            ztab = cload(d_zt, [128, 2, K, 1], bf16, "ztab", nc.scalar)
            id1 = cload(d_id1, [1, 1], f32, "id1", nc.scalar)
            gt = cload(d_gt, [128, 2, K, DIM], bf16, "gt", nc.gpsimd)
            at = cload(d_at, [TS, 4, TS], bf16, "at", nc.gpsimd)
            pt = cload(d_pt, [TS, 4, TS], bf16, "pt", nc.scalar)
            p4t = cload(d_p4, [TS, DSR], bf16, "p4t", nc.scalar)
            idt = cload(d_idt, [TS, TS], f32, "idt", nc.gpsimd)
            mt = cload(d_mt, [NF, NF], f32, "mt", nc.scalar)
            pmask = cload(d_pm, [TS, NT], f32, "pmask", nc.scalar)

            # persistent SBUF buffers
            oh = spool.tile([128, 2, 2, IDSP], bf16)     # onehot [vc, blk, pos]
            h_sb = spool.tile([TS, NT_LOC, DIM], bf16)   # local h_proj
            ztl = spool.tile([TS, NT], bf16)             # z per tile col
            zrow = spool.tile([1, 2, IDSP], bf16)           # remote z row
            u_sb = spool.tile([TS, NT, 4], f32)
            den = spool.tile([TS, NT], f32)
            q_sb = spool.tile([TS, NT, 4], f32)
            psi = spool.tile([TS, NT, NF], f32)
            w_sb = spool.tile([NF, 4], f32)
            wq = spool.tile([NF, 4], f32)
            den2 = spool.tile([TS, NT_LOC], f32)
            fsum = spool.tile([TS, NT_LOC], f32)
            fp = spool.tile([TS, NT_LOC, 4], f32)
            F4 = spool.tile([TS, NT_LOC, 4, DSR], bf16)
            g2 = spool.tile([TS, NT_LOC, DSR], bf16)

            # ---- 1. one-hot build: broadcast ids down 128 partitions, compare to iota
            SL = [(0, 512), (512, 512), (1024, 512), (1536, 512), (2048, 128)]
            for blk in range(2):
                for (s0, w) in SL:
                    idb = pbig.tile([128, 512], f32, tag="a")
                    nc.tensor.matmul(idb[:, :w], lhsT=o128[:, :],
                                     rhs=ids[:, blk, s0:s0 + w],
                                     start=True, stop=True)
                    for vc in range(2):
                        nc.vector.tensor_scalar(
                            out=oh[:, vc, blk, s0:s0 + w], in0=idb[:, :w],
                            scalar1=iota[:, vc:vc + 1], scalar2=None, op0=ISEQ)

            # ---- 2. z rows (one-hot moving), PE-transpose into ztl columns
            SLZ = [(0, 512), (512, 512), (1024, 512), (1536, 512), (2048, 115)]
            for blk in range(2):
                for (s0, w) in SLZ:
                    zr_ps = pseq.tile([1, 512], f32, tag="s")
                    first = True
                    for k in range(K):
                        for vc in range(2):
                            nc.tensor.matmul(
                                zr_ps[:, :w], lhsT=ztab[:, vc, k, :],
                                rhs=oh[:, vc, blk, s0 + k: s0 + k + w],
                                start=first, stop=(k == K - 1 and vc == 1))
                            first = False
                    nc.vector.tensor_copy(zrow[:, blk, s0:s0 + w], zr_ps[:, :w])
            zdram = dpool.tile([2, NT_LOC, TS], bf16)
            nc.scalar.dma_start(zdram[:, :, :],
                                zrow[:, :, 0:TS * NT_LOC].rearrange(
                                    "o blk (t p) -> (o blk) t p", p=TS))
            nc.sync.dma_start(
                ztl[:].rearrange("p (blk t) -> p blk t", blk=2),
                zdram[:].rearrange("blk t p -> p blk t"))

            # ---- 3. block-scale scores (batched over all 36 tiles)
            sraw = psr.tile([TS, 4, NT], f32, tag="sr")
            for k in range(4):
                nc.tensor.matmul(sraw[:, k, :], lhsT=at[:, k, :], rhs=ztl[:],
                                 start=True, stop=True)

            # ---- 4a. first half of gcp (PE) — overlaps the softmax/psi DVE work
            def gcp_tile(t):
                h_ps0 = pbig.tile([128, DIM], f32, tag="a")
                h_ps = h_ps0[:TS, :]
                first = True
                for k in range(K):
                    for vc in range(2):
                        lhs = oh[:, vc, 0, TS * t + k: TS * t + k + TS]
                        nc.tensor.matmul(h_ps[:], lhsT=lhs, rhs=gt[:, vc, k, :],
                                         start=first,
                                         stop=(k == K - 1 and vc == 1))
                        first = False
                nc.scalar.copy(h_sb[:, t, :], h_ps[:])

            for t in range(0, 9):
                gcp_tile(t)

            # ---- 4b. softmax over scales + psi features (DVE/ACT, under gcp)
            nc.scalar.activation(u_sb[:].rearrange("p t k -> p k t"), sraw[:],
                                 EXP, bias=0.0, scale=1.0)
            nc.vector.tensor_reduce(den[:], u_sb[:], axis=X, op=ADD)
            nc.vector.reciprocal(den[:], den[:])
            nc.vector.tensor_tensor(
                q_sb[:], u_sb[:],
                den[:].unsqueeze(2).broadcast_to([TS, NT, 4]), op=MUL)

            nc.vector.memset(psi[:, :, 0:1], 1.0)
            nc.vector.tensor_copy(psi[:, :, 1:4], q_sb[:, :, 0:3])
            deg2 = [(4, 1, 1), (5, 1, 2), (6, 1, 3), (7, 2, 2), (8, 2, 3),
                    (9, 3, 3)]
            deg3 = [(10, 1, 4), (11, 1, 5), (12, 1, 6), (13, 1, 7), (14, 1, 8),
                    (15, 1, 9), (16, 2, 7), (17, 2, 8), (18, 2, 9), (19, 3, 9)]
            for (o, a, bb) in deg2 + deg3:
                nc.vector.tensor_tensor(psi[:, :, o:o + 1], psi[:, :, a:a + 1],
                                        psi[:, :, bb:bb + 1], op=MUL)
            nc.vector.tensor_tensor(
                psi[:], psi[:],
                pmask[:].unsqueeze(2).broadcast_to([TS, NT, NF]), op=MUL)
            vaug = spool.tile([TS, NT, 4], f32)
            nc.vector.tensor_copy(vaug[:, :, 0:3], q_sb[:, :, 0:3])
            nc.vector.memset(vaug[:, :, 3:4], 1.0)

            # ---- 5. W = sum_j psi_j [q_j(1:3) | 1]
            wps = pseq.tile([NF, 4], f32, tag="s")
            for t in range(NT):
                nc.tensor.matmul(wps[:], lhsT=psi[:, t, :], rhs=vaug[:, t, :],
                                 start=(t == 0), stop=(t == NT - 1))
            nc.vector.tensor_copy(w_sb[:], wps[:])
            wqp = pseq.tile([NF, 4], f32, tag="s")
            nc.tensor.matmul(wqp[:], lhsT=mt[:, :], rhs=w_sb[:],
                             start=True, stop=True)
            nc.vector.tensor_copy(wq[:], wqp[:])

            # ---- 6. out_aug = psi (M W) for local tiles
            oa = psr.tile([TS, NT_LOC * 4], f32, tag="sr")
            for t in range(NT_LOC):
                pT = pseq.tile([NF, TS], f32, tag="s")
                nc.tensor.transpose(pT[:], in_=psi[:, t, :], identity=idt[:, :])
                pTs = wpool.tile([NF, TS], f32, tag="pTs")
                nc.scalar.copy(pTs[:], pT[:])
                nc.tensor.matmul(oa[:, 4 * t: 4 * t + 4], lhsT=pTs[:],
                                 rhs=wq[:], start=True, stop=True)

            # ---- 7a. second half of gcp — overlaps the fscore/F DVE work
            for t in range(9, NT_LOC):
                gcp_tile(t)

            # ---- 7b. consensus scores f (f4 = 1 - f1 - f2 - f3)
            oa3 = oa[:].rearrange("p (t k) -> p t k", k=4)
            nc.vector.tensor_scalar_max(den2[:], oa3[:, :, 3], 1e-20)
            nc.vector.reciprocal(den2[:], den2[:])
            nc.vector.tensor_tensor(
                fp[:, :, 0:3], oa3[:, :, 0:3],
                den2[:].unsqueeze(2).broadcast_to([TS, NT_LOC, 3]), op=MUL)
            nc.vector.tensor_reduce(fsum[:], fp[:, :, 0:3], axis=X, op=ADD)
            nc.vector.tensor_scalar(out=fp[:, :, 3:4].rearrange("p t k -> p (t k)"),
                                    in0=fsum[:], scalar1=-1.0, scalar2=1.0,
                                    op0=MUL, op1=ADD)
            nc.vector.tensor_tensor(
                F4[:],
                fp[:].unsqueeze(3).broadcast_to([TS, NT_LOC, 4, DSR]),
                p4t[:].unsqueeze(1).unsqueeze(1).broadcast_to(
                    [TS, NT_LOC, 4, DSR]),
                op=MUL)

            # ---- 8. Gall = sum_k Pt_k @ F_k; tiles ride the free dim,
            # one stationary per k, k accumulated in PSUM
            ga0 = pbig.tile([128, 510], f32, tag="a")
            ga = ga0[:TS, :]
            gb = pseq.tile([TS, DSR], f32, tag="s")
            for k in range(4):
                nc.tensor.matmul(ga[:], lhsT=pt[:, k, :],
                                 rhs=F4[:, 0:17, k, :],
                                 start=(k == 0), stop=(k == 3))
                nc.tensor.matmul(gb[:], lhsT=pt[:, k, :],
                                 rhs=F4[:, 17, k, :],
                                 start=(k == 0), stop=(k == 3))
            nc.vector.tensor_copy(
                g2[:, 0:17, :], ga[:].rearrange("p (t c) -> p t c", c=DSR))
            nc.vector.tensor_copy(g2[:, 17, :], gb[:])

            # ---- 9. fused mixing + 4x downsample pool, grouped DMA out
            osb = spool.tile([DSR, NT_LOC, DIM], f32)
            outv = d_out.ap().rearrange("(t p) d -> p t d", p=DSR)
            groups = [(0, 5, nc.sync), (5, 10, nc.gpsimd), (10, 14, nc.scalar),
                      (14, 18, nc.sync)]
            for (g0, g1, eng) in groups:
                for t in range(g0, g1):
                    ods = pods.tile([DSR, DIM], f32, tag="ods")
                    nc.tensor.matmul(ods[:], lhsT=g2[:, t, :],
                                     rhs=h_sb[:, t, :], start=True, stop=True)
                    if t % 2 == 0:
                        nc.scalar.copy(osb[:, t, :], ods[:])
                    else:
                        nc.vector.tensor_copy(osb[:, t, :], ods[:])
                eng.dma_start(outv[:, g0:g1, :], osb[:, g0:g1, :])

    nc.compile()
    return nc


def _host_prep(emb, conv_w, conv_b, proj_w, proj_b, score_w, score_b):
    G = np.stack([(emb * conv_w[:, k][None, :]) @ proj_w.T for k in range(K)])
    C = conv_b @ proj_w.T + proj_b
    G[0] += C
    g = G @ score_w                      # [4, 256]
    gt = np.zeros((128, 2, K, DIM), np.float32)
    zt = np.zeros((128, 2, K, 1), np.float32)
    for vc in range(2):
        for k in range(K):
            gt[:, vc, k, :] = G[k][128 * vc:128 * vc + 128]
            zt[:, vc, k, 0] = g[k][128 * vc:128 * vc + 128]
    at, pt, p4 = _pool_mats()
    M = _poly_M()
    iota = np.stack([np.arange(128, dtype=np.float32),
                     np.arange(128, 256, dtype=np.float32)], axis=1)
    consts = {
        "gt": _bf(gt), "ztab": _bf(zt), "at": _bf(at), "pt": _bf(pt),
        "p4t": _bf(p4), "idt": np.eye(TS, dtype=np.float32),
        "id1": np.ones((1, 1), np.float32), "ones128": _bf(np.ones((1, 128))),
        "iota": iota,
        "mt": np.ascontiguousarray(M.T),
    }
    return consts, np.float32(score_b)


def _core_inputs(x_row, hi):
    """ids [1,2,IDSP] bf16 and pmask [TS,NT] f32 for core half hi."""
    idsp = np.full(L + K - 1, -1.0, np.float32)
    idsp[:N] = x_row.astype(np.float32)

    def block(start):
        out = np.full(IDSP, -1.0, np.float32)
        lo = start
        hhi = min(start + IDS_LEN, L + K - 1)
        out[:hhi - lo] = idsp[lo:hhi]
        return out

    o_loc = 0 if hi == 0 else L - BLK          # 0 or 1944? no: 2040
    o_loc = 0 if hi == 0 else 2040
    o_rem = 2040 if hi == 0 else 0
    ids = np.stack([block(o_loc), block(o_rem)])[None]  # [1,2,IDSP]

    pm = np.zeros((TS, NT), np.float32)
    for j in range(NT):
        base = (o_loc if j < NT_LOC else o_rem) + TS * (j % NT_LOC)
        gpos = base + np.arange(TS)
        valid = gpos < L
        if hi == 0 and j == NT_LOC:          # remote tile at 2040 duplicates local t=17
            valid &= False
        if hi == 1 and j == NT - 1:          # remote tile at 2040 duplicates local t=0
            valid &= False
        pm[:, j] = valid.astype(np.float32)
    return _bf(ids), pm


def kernel(x, emb, conv_w, conv_b, proj_w, proj_b, score_w, score_b):
    from concourse import bass_utils

    x = np.asarray(x)
    emb = np.asarray(emb, np.float32)
    conv_w = np.asarray(conv_w, np.float32)
    conv_b = np.asarray(conv_b, np.float32)
    proj_w = np.asarray(proj_w, np.float32)
    proj_b = np.asarray(proj_b, np.float32)
    score_w = np.asarray(score_w, np.float32)
    score_b = np.float32(np.asarray(score_b))

    if "nc" not in _CACHE:
        _CACHE["nc"] = _build_module()
    nc = _CACHE["nc"]

    consts, sb = _host_prep(emb, conv_w, conv_b, proj_w, proj_b,
                            score_w, score_b)
    # score_b is folded as exp bias -> bake into ztab? No: it is a bias on
    # scores_raw. We add it on host into the A_bs result via ztab offset:
    # scores_raw = A_bs z + score_b. Instead fold into z: z' = z + score_b
    # would shift all scales equally -> softmax invariant? No: A_bs averages
    # z, so adding score_b to every z entry adds score_b to every score --
    # softmax over k is invariant to a common shift. pad positions get
    # masked anyway. So score_b can be DROPPED entirely.
    del sb

    in_maps = []
    for c in range(NC):
        bi, hi = divmod(c, 2)
        ids, pm = _core_inputs(x[bi], hi)
        m = dict(consts)
        m["ids"] = ids
        m["pmask"] = pm
        in_maps.append(m)

    res = bass_utils.run_bass_kernel_spmd(
        nc, in_maps, core_ids=list(range(NC)), trace=_CACHE.get("trace", False))
    _CACHE["last_exec_ns"] = res.exec_time_ns

    out = np.empty((B, N // DS, DIM), np.float32)
    for c in range(NC):
        bi, hi = divmod(c, 2)
        r = res.results[c]["out"]
        if hi == 0:
            out[bi, 0:540] = r[0:540]
        else:
            out[bi, 540:1024] = r[30:514]
    return out
